# revision 6
# baseline (speedup 1.0000x reference)
"""AdaptiveFusionLayer Trainium2 kernel — 8-core data-parallel, gather-free.

Math (derived from the reference):
  u2 = W @ a[D:]                               # [D]
  v  = concat([F_1 @ u2, F_2 @ u2, F_3 @ u2])  # [3N]  (block layout)
  e[n, :] = softmax_k(v[3n + k])               # Wh1 cancels (softmax shift-invariance)
  wh = e0*F_1 + e1*F_2 + e2*F_3                # row-wise
  out = relu(wh @ W) + F_0

Sharding: node-blocks of B = N/8 rows per core.  The score entries core c
needs, v[3Bc : 3B(c+1)), are exactly blocks q = 3c+r (r=0..2) of v, where
block q is F_{q//8 + 1}[ (q%8)·B : (q%8+1)·B ] @ u2.  The host hands core c
those three row-blocks as extra inputs g0..g2, so no collective is needed:
every core computes exactly the scores it consumes.
"""

import numpy as np

N, D, NCORES = 50000, 768, 8
B_FULL = N // NCORES           # 6250 real nodes per core
T_FULL = (B_FULL + 127) // 128  # 49 node-tiles (padded)


def build_kernel(B=B_FULL, T=T_FULL, stage=99):
    from concourse import bass, bacc, tile, mybir

    f32 = mybir.dt.float32
    bf16 = mybir.dt.bfloat16
    Alu = mybir.AluOpType
    Act = mybir.ActivationFunctionType

    BP = T * 128                # padded nodes per core
    VPAD = 3 * BP               # padded score vector length
    FULL_T = B // 128           # full 128-tiles of real nodes
    TAIL = B - FULL_T * 128     # real nodes in the last tile
    NQ = D // 128               # 6 chunks of the feature dim

    nc = bacc.Bacc(None, target_bir_lowering=False, debug=False)

    f = [nc.declare_dram_parameter(f"f{j}", [BP, D], f32, isOutput=False)
         for j in range(4)]
    g = [nc.declare_dram_parameter(f"g{r}", [BP, D], f32, isOutput=False)
         for r in range(3)]
    w = nc.declare_dram_parameter("w", [D, D], f32, isOutput=False)
    a2row = nc.declare_dram_parameter("a2row", [1, D], f32, isOutput=False)
    eye = nc.declare_dram_parameter("eye", [128, 128], bf16, isOutput=False)
    out = nc.declare_dram_parameter("out", [BP, D], f32, isOutput=True)

    with tile.TileContext(nc) as tc:
        with (
            tc.tile_pool(name="const", bufs=1) as constp,
            tc.tile_pool(name="gpool", bufs=4) as gpool,
            tc.tile_pool(name="fpool", bufs=3) as fpool,
            tc.tile_pool(name="scr", bufs=2) as scrp,
            tc.tile_pool(name="whpool", bufs=3) as whpool,
            tc.tile_pool(name="psum", bufs=2, space="PSUM") as psump,
            tc.tile_pool(name="dram", bufs=1, space="DRAM") as dramp,
        ):
            def dummy_out():
                dummy = whpool.tile([128, D], f32, tag="res", name="dummy")
                nc.vector.memset(dummy[:], 0.0)
                for t in range(T):
                    nc.sync.dma_start(out[t * 128:(t + 1) * 128, :], dummy[:])

            # ---------------- setup ----------------
            w_sb = constp.tile([128, NQ * D], f32)
            for q in range(NQ):
                nc.sync.dma_start(w_sb[:, q * D:(q + 1) * D],
                                  w[q * 128:(q + 1) * 128, :])
            w_bf = constp.tile([128, NQ * D], bf16)
            for q in range(NQ):
                nc.scalar.copy(w_bf[:, q * D:(q + 1) * D],
                               w_sb[:, q * D:(q + 1) * D])
            a2_sb = constp.tile([1, D], f32)
            nc.sync.dma_start(a2_sb[:], a2row[:])
            eye_sb = constp.tile([128, 128], bf16)
            nc.sync.dma_start(eye_sb[:], eye[:])
            ones_sb = constp.tile([1, 128], f32)
            nc.vector.memset(ones_sb[:], 1.0)

            # a2 broadcast across partitions via PE outer product
            bc_ps = psump.tile([128, D], f32, tag="wh")
            nc.tensor.matmul(bc_ps[:, 0:512], ones_sb[:], a2_sb[:, 0:512],
                             start=True, stop=True)
            nc.tensor.matmul(bc_ps[:, 512:D], ones_sb[:], a2_sb[:, 512:D],
                             start=True, stop=True)
            a2b_sb = constp.tile([128, D], f32)
            nc.scalar.copy(a2b_sb[:], bc_ps[:, 0:D])

            # u2 = W @ a2 : chunk q on partitions -> u2col[:, q]
            u2col = constp.tile([128, NQ], f32)
            for q in range(NQ):
                ttr_scr = scrp.tile([128, D], f32, tag="ttr", name="ttr_scr")
                nc.vector.affine_mul_reduce(
                    out=ttr_scr[:], accum_out=u2col[:, q:q + 1],
                    in0=w_sb[:, q * D:(q + 1) * D], in1=a2b_sb[:],
                    scale=1.0, bias=0.0)
            # u2col [128, NQ] -> u2row [1, D]  (d = 128 q + p)
            u2row = constp.tile([1, D], f32)
            for q in range(NQ):
                nc.sync.dma_start(
                    u2row[:, q * 128:(q + 1) * 128]
                    .rearrange("o (p z) -> o p z", z=1),
                    u2col[:, q:q + 1])
            # u2 broadcast across partitions
            u2b_ps = psump.tile([128, D], f32, tag="wh")
            nc.tensor.matmul(u2b_ps[:, 0:512], ones_sb[:], u2row[:, 0:512],
                             start=True, stop=True)
            nc.tensor.matmul(u2b_ps[:, 512:D], ones_sb[:], u2row[:, 512:D],
                             start=True, stop=True)
            u2b_sb = constp.tile([128, D], f32)
            nc.scalar.copy(u2b_sb[:], u2b_ps[:, 0:D])

            done = stage < 2
            if done:
                dummy_out()

            # ---------------- phase A: scores ----------------
            if not done:
                sig_sb = constp.tile([128, 3 * T], f32)   # col r*T + t
                for r in range(3):
                    for t in range(T):
                        gt = gpool.tile([128, D], f32, tag="g", name="gt")
                        nc.sync.dma_start(gt[:], g[r][t * 128:(t + 1) * 128, :])
                        ttr_scr = scrp.tile([128, D], f32, tag="ttr",
                                            name="ttr_scr")
                        nc.vector.affine_mul_reduce(
                            out=ttr_scr[:],
                            accum_out=sig_sb[:, r * T + t: r * T + t + 1],
                            in0=gt[:], in1=u2b_sb[:], scale=1.0, bias=0.0)

                # scatter scores to DRAM: vloc[r*B + 128 t + p]
                vloc = dramp.tile([VPAD], f32)
                for r in range(3):
                    if FULL_T:
                        nc.sync.dma_start(
                            vloc[r * B: r * B + FULL_T * 128]
                            .rearrange("(t p) -> p t", p=128),
                            sig_sb[:, r * T: r * T + FULL_T])
                    if TAIL:
                        nc.sync.dma_start(
                            vloc[r * B + FULL_T * 128: (r + 1) * B]
                            .rearrange("(p o) -> p o", o=1),
                            sig_sb[0:TAIL, r * T + FULL_T: r * T + FULL_T + 1])
                # zero the padded tail so exp() stays finite
                if VPAD > 3 * B:
                    zpad = constp.tile([VPAD - 3 * B, 1], f32)
                    nc.vector.memset(zpad[:], 0.0)
                    nc.sync.dma_start(
                        vloc[3 * B: VPAD].rearrange("(p o) -> p o", o=1),
                        zpad[:])
                if stage < 3:
                    dummy_out()
                    done = True

            # ---------------- softmax ----------------
            if not done:
                # L[p, 3 tau + k] = vloc[384 tau + 3 p + k]
                L_sb = constp.tile([128, 3 * T], f32)
                nc.sync.dma_start(
                    L_sb.rearrange("p (t k) -> p t k", k=3),
                    vloc[:].rearrange("(t p k) -> p t k", p=128, k=3))
                P_sb = constp.tile([128, 3 * T], f32)
                nc.scalar.activation(P_sb[:], L_sb[:], Act.Exp)
                S_sb = constp.tile([128, T], f32)
                nc.vector.tensor_reduce(
                    S_sb[:], P_sb.rearrange("p (t k) -> p t k", k=3),
                    axis=mybir.AxisListType.X, op=Alu.add)
                R_sb = constp.tile([128, T], f32)
                nc.vector.reciprocal(R_sb[:], S_sb[:])
                E_sb = constp.tile([128, 3 * T], f32)  # col k*T + tau
                for k in range(3):
                    nc.vector.tensor_mul(
                        E_sb[:, k * T:(k + 1) * T],
                        P_sb.rearrange("p (t k) -> p t k", k=3)[:, :, k],
                        R_sb[:])
                if stage < 4:
                    dummy_out()
                    done = True

            # ---------------- phase B ----------------
            if not done:
                for t in range(T):
                    f0t = fpool.tile([128, D], f32, tag="f0", name="f0t")
                    nc.sync.dma_start(f0t[:], f[0][t * 128:(t + 1) * 128, :])
                    fb = []
                    for j in (1, 2, 3):
                        fjt = fpool.tile([128, D], f32, tag=f"f{j}",
                                         name="fjt")
                        nc.sync.dma_start(fjt[:],
                                          f[j][t * 128:(t + 1) * 128, :])
                        fjb = fpool.tile([128, D], bf16, tag=f"f{j}b",
                                         name="fjb")
                        nc.scalar.copy(fjb[:], fjt[:])
                        fb.append(fjb)
                    dk = []
                    for k in range(3):
                        dkt = scrp.tile([128, 128], bf16, tag=f"d{k}",
                                        name="dkt")
                        nc.vector.tensor_scalar_mul(
                            dkt[:], eye_sb[:],
                            E_sb[:, k * T + t: k * T + t + 1])
                        dk.append(dkt)
                    # whT chunks: psum[p, 128 q + n] = wh[node n, d = 128 q + p]
                    whp = psump.tile([128, D], f32, tag="wh")
                    for q in range(NQ):
                        for k in range(3):
                            nc.tensor.matmul(
                                whp[:, q * 128:(q + 1) * 128],
                                fb[k][:, q * 128:(q + 1) * 128], dk[k][:],
                                start=(k == 0), stop=(k == 2))
                    whs = whpool.tile([128, D], bf16, tag="whs")
                    nc.scalar.copy(whs[:], whp[:, 0:D])
                    # out = wh @ W : accumulate over d-chunks
                    outp = psump.tile([128, D], f32, tag="out")
                    for q in range(NQ):
                        nc.tensor.matmul(
                            outp[:, 0:512], whs[:, q * 128:(q + 1) * 128],
                            w_bf[:, q * D: q * D + 512],
                            start=(q == 0), stop=(q == NQ - 1))
                    for q in range(NQ):
                        nc.tensor.matmul(
                            outp[:, 512:D], whs[:, q * 128:(q + 1) * 128],
                            w_bf[:, q * D + 512:(q + 1) * D],
                            start=(q == 0), stop=(q == NQ - 1))
                    rel = whpool.tile([128, D], f32, tag="rel")
                    nc.scalar.activation(rel[:], outp[:, 0:D], Act.Relu)
                    res = whpool.tile([128, D], f32, tag="res")
                    nc.vector.tensor_add(res[:], rel[:], f0t[:])
                    nc.sync.dma_start(out[t * 128:(t + 1) * 128, :], res[:])

    nc.compile()
    return nc


def make_in_maps(F_0, F_1, F_2, F_3, W, a, B=B_FULL, T=T_FULL, ncores=NCORES):
    import ml_dtypes
    BP = T * 128
    Fs = [np.asarray(x, np.float32) for x in (F_0, F_1, F_2, F_3)]
    d = Fs[0].shape[1]
    wf = np.ascontiguousarray(np.asarray(W, np.float32))
    a2 = np.ascontiguousarray(np.asarray(a, np.float32)[d:, 0]).reshape(1, d)
    eye = np.eye(128, dtype=ml_dtypes.bfloat16)
    in_maps = []
    for c in range(ncores):
        m = {"w": wf, "a2row": a2, "eye": eye}
        for j in range(4):
            fj = np.zeros((BP, d), np.float32)
            fj[:B] = Fs[j][c * B:(c + 1) * B]
            m[f"f{j}"] = fj
        for r in range(3):
            q = 3 * c + r
            jj, b = q // ncores, q % ncores
            gr = np.zeros((BP, d), np.float32)
            gr[:B] = Fs[jj + 1][b * B:(b + 1) * B]
            m[f"g{r}"] = gr
        in_maps.append(m)
    return in_maps


_NC_CACHE = {}


def kernel(F_0, F_1, F_2, F_3, W, a):
    from concourse.bass_utils import run_bass_kernel_spmd
    if "nc" not in _NC_CACHE:
        _NC_CACHE["nc"] = build_kernel()
    nc = _NC_CACHE["nc"]
    in_maps = make_in_maps(F_0, F_1, F_2, F_3, W, a)
    res = run_bass_kernel_spmd(nc, in_maps, core_ids=list(range(NCORES)))
    out = np.concatenate(
        [res.results[c]["out"][:B_FULL] for c in range(NCORES)], axis=0)
    return np.ascontiguousarray(out, np.float32)


if __name__ == "__main__":
    nc = build_kernel()
    print("build ok")


# revision 7
# speedup vs baseline: 1.1199x; 1.1199x over previous
"""AdaptiveFusionLayer Trainium2 kernel — 8-core data-parallel, gather-free.

Math (derived from the reference):
  u2 = W @ a[D:]                               # [D]
  v  = concat([F_1 @ u2, F_2 @ u2, F_3 @ u2])  # [3N]  (block layout)
  e[n, :] = softmax_k(v[3n + k])               # Wh1 cancels (softmax shift-invariance)
  wh = e0*F_1 + e1*F_2 + e2*F_3                # row-wise
  out = relu(wh @ W) + F_0

Sharding: node-blocks of B = N/8 rows per core.  The score entries core c
needs, v[3Bc : 3B(c+1)), are exactly blocks q = 3c+r (r=0..2) of v, where
block q is F_{q//8 + 1}[ (q%8)·B : (q%8+1)·B ] @ u2.  The host hands core c
those three row-blocks as extra inputs g0..g2, so no collective is needed:
every core computes exactly the scores it consumes.
"""

import numpy as np

N, D, NCORES = 50000, 768, 8
B_FULL = N // NCORES           # 6250 real nodes per core
T_FULL = (B_FULL + 127) // 128  # 49 node-tiles (padded)


def build_kernel(B=B_FULL, T=T_FULL, stage=99):
    from concourse import bass, bacc, tile, mybir

    f32 = mybir.dt.float32
    bf16 = mybir.dt.bfloat16
    Alu = mybir.AluOpType
    Act = mybir.ActivationFunctionType

    BP = T * 128                # padded nodes per core
    VPAD = 3 * BP               # padded score vector length
    FULL_T = B // 128           # full 128-tiles of real nodes
    TAIL = B - FULL_T * 128     # real nodes in the last tile
    NQ = D // 128               # 6 chunks of the feature dim

    nc = bacc.Bacc(None, target_bir_lowering=False, debug=False)

    f = [nc.declare_dram_parameter(f"f{j}", [BP, D], f32, isOutput=False)
         for j in range(4)]
    g = [nc.declare_dram_parameter(f"g{r}", [BP, D], f32, isOutput=False)
         for r in range(3)]
    w = nc.declare_dram_parameter("w", [D, D], f32, isOutput=False)
    a2row = nc.declare_dram_parameter("a2row", [1, D], f32, isOutput=False)
    eye = nc.declare_dram_parameter("eye", [128, 128], f32, isOutput=False)
    out = nc.declare_dram_parameter("out", [BP, D], f32, isOutput=True)

    with tile.TileContext(nc) as tc:
        with (
            tc.tile_pool(name="const", bufs=1) as constp,
            tc.tile_pool(name="gpool", bufs=6) as gpool,
            tc.tile_pool(name="fpool", bufs=4) as fpool,
            tc.tile_pool(name="scr", bufs=2) as scrp,
            tc.tile_pool(name="whpool", bufs=3) as whpool,
            tc.tile_pool(name="psum", bufs=2, space="PSUM") as psump,
            tc.tile_pool(name="dram", bufs=1, space="DRAM") as dramp,
        ):
            def dummy_out():
                dummy = whpool.tile([128, D], f32, tag="res", name="dummy")
                nc.vector.memset(dummy[:], 0.0)
                for t in range(T):
                    nc.sync.dma_start(out[t * 128:(t + 1) * 128, :], dummy[:])

            # ---------------- setup ----------------
            w_sb = constp.tile([128, NQ * D], f32)
            for q in range(NQ):
                nc.sync.dma_start(w_sb[:, q * D:(q + 1) * D],
                                  w[q * 128:(q + 1) * 128, :])
            w_bf = constp.tile([128, NQ * D], bf16)
            for q in range(NQ):
                nc.scalar.copy(w_bf[:, q * D:(q + 1) * D],
                               w_sb[:, q * D:(q + 1) * D])
            a2_sb = constp.tile([1, D], f32)
            nc.sync.dma_start(a2_sb[:], a2row[:])
            eye_f32 = constp.tile([128, 128], f32)
            nc.sync.dma_start(eye_f32[:], eye[:])
            eye_sb = constp.tile([128, 128], bf16)
            nc.scalar.copy(eye_sb[:], eye_f32[:])
            ones_sb = constp.tile([1, 128], f32)
            nc.vector.memset(ones_sb[:], 1.0)

            # a2 broadcast across partitions via PE outer product
            bc_ps = psump.tile([128, D], f32, tag="wh")
            nc.tensor.matmul(bc_ps[:, 0:512], ones_sb[:], a2_sb[:, 0:512],
                             start=True, stop=True)
            nc.tensor.matmul(bc_ps[:, 512:D], ones_sb[:], a2_sb[:, 512:D],
                             start=True, stop=True)
            a2b_sb = constp.tile([128, D], f32)
            nc.scalar.copy(a2b_sb[:], bc_ps[:, 0:D])

            # u2 = W @ a2 : chunk q on partitions -> u2col[:, q]
            u2col = constp.tile([128, NQ], f32)
            for q in range(NQ):
                ttr_scr = scrp.tile([128, D], f32, tag="ttr", name="ttr_scr")
                nc.vector.affine_mul_reduce(
                    out=ttr_scr[:], accum_out=u2col[:, q:q + 1],
                    in0=w_sb[:, q * D:(q + 1) * D], in1=a2b_sb[:],
                    scale=1.0, bias=0.0)
            # u2col [128, NQ] -> u2row [1, D]  (d = 128 q + p)
            u2row = constp.tile([1, D], f32)
            for q in range(NQ):
                nc.sync.dma_start(
                    u2row[:, q * 128:(q + 1) * 128]
                    .rearrange("o (p z) -> o p z", z=1),
                    u2col[:, q:q + 1])
            # u2 broadcast across partitions
            u2b_ps = psump.tile([128, D], f32, tag="wh")
            nc.tensor.matmul(u2b_ps[:, 0:512], ones_sb[:], u2row[:, 0:512],
                             start=True, stop=True)
            nc.tensor.matmul(u2b_ps[:, 512:D], ones_sb[:], u2row[:, 512:D],
                             start=True, stop=True)
            u2b_sb = constp.tile([128, D], f32)
            nc.scalar.copy(u2b_sb[:], u2b_ps[:, 0:D])

            done = stage < 2
            if done:
                dummy_out()

            # ---------------- phase A: scores ----------------
            if not done:
                sig_sb = constp.tile([128, 3 * T], f32)   # col r*T + t
                for r in range(3):
                    for t in range(T):
                        gt = gpool.tile([128, D], f32, tag="g", name="gt")
                        nc.sync.dma_start(gt[:], g[r][t * 128:(t + 1) * 128, :])
                        ttr_scr = scrp.tile([128, D], f32, tag="ttr",
                                            name="ttr_scr")
                        nc.vector.affine_mul_reduce(
                            out=ttr_scr[:],
                            accum_out=sig_sb[:, r * T + t: r * T + t + 1],
                            in0=gt[:], in1=u2b_sb[:], scale=1.0, bias=0.0)

                # transpose scores to [t, p] and scatter contiguously:
                # vloc[r*B + 128 t + p]
                vloc = dramp.tile([VPAD], f32)
                for r in range(3):
                    sigT_ps = psump.tile([T, 128], f32, tag="wh",
                                         name="sigT_ps")
                    nc.tensor.transpose(sigT_ps[:],
                                        sig_sb[:, r * T:(r + 1) * T],
                                        eye_f32[:])
                    sigT_sb = scrp.tile([T, 128], f32, tag="sigT",
                                        name="sigT_sb")
                    nc.scalar.copy(sigT_sb[:], sigT_ps[:])
                    if FULL_T:
                        nc.sync.dma_start(
                            vloc[r * B: r * B + FULL_T * 128]
                            .rearrange("(t p) -> t p", p=128),
                            sigT_sb[0:FULL_T, :])
                    if TAIL:
                        nc.sync.dma_start(
                            vloc[r * B + FULL_T * 128: (r + 1) * B]
                            .rearrange("(o p) -> o p", o=1),
                            sigT_sb[FULL_T:FULL_T + 1, 0:TAIL])
                # zero the padded tail so exp() stays finite
                if VPAD > 3 * B:
                    zpad = constp.tile([VPAD - 3 * B, 1], f32)
                    nc.vector.memset(zpad[:], 0.0)
                    nc.sync.dma_start(
                        vloc[3 * B: VPAD].rearrange("(p o) -> p o", o=1),
                        zpad[:])
                if stage < 3:
                    dummy_out()
                    done = True

            # ---------------- softmax ----------------
            if not done:
                # V2[t, c] = vloc[384 t + c]; L_k[p, t] = V2[t, 3p + k]
                V2_sb = constp.tile([T, 3 * 128], f32)
                nc.sync.dma_start(
                    V2_sb[:], vloc[:].rearrange("(t c) -> t c", c=384))
                Ps = []
                for k in range(3):
                    Lk_ps = psump.tile([128, T], f32, tag="wh", name="Lk_ps")
                    nc.tensor.transpose(
                        Lk_ps[:],
                        V2_sb.rearrange("t (p k) -> t p k", k=3)[:, :, k],
                        eye_f32[0:T, 0:T])
                    Pk = constp.tile([128, T], f32, name=f"P{k}_sb",
                                     tag=f"P{k}")
                    nc.scalar.activation(Pk[:], Lk_ps[:], Act.Exp)
                    Ps.append(Pk)
                S_sb = constp.tile([128, T], f32)
                nc.vector.tensor_add(S_sb[:], Ps[0][:], Ps[1][:])
                S2_sb = constp.tile([128, T], f32)
                nc.vector.tensor_add(S2_sb[:], S_sb[:], Ps[2][:])
                R_sb = constp.tile([128, T], f32)
                nc.vector.reciprocal(R_sb[:], S2_sb[:])
                E_sb = constp.tile([128, 3 * T], f32)  # col k*T + tau
                for k in range(3):
                    nc.vector.tensor_mul(
                        E_sb[:, k * T:(k + 1) * T], Ps[k][:], R_sb[:])
                if stage < 4:
                    dummy_out()
                    done = True

            # ---------------- phase B ----------------
            if not done:
                for t in range(T):
                    f0t = fpool.tile([128, D], f32, tag="f0", name="f0t")
                    nc.scalar.dma_start(f0t[:], f[0][t * 128:(t + 1) * 128, :])
                    fb = []
                    for j in (1, 2, 3):
                        fjt = fpool.tile([128, D], f32, tag=f"f{j}",
                                         name="fjt")
                        eng = nc.scalar if j == 2 else nc.sync
                        eng.dma_start(fjt[:],
                                      f[j][t * 128:(t + 1) * 128, :])
                        fjb = fpool.tile([128, D], bf16, tag=f"f{j}b",
                                         name="fjb")
                        nc.scalar.copy(fjb[:], fjt[:])
                        fb.append(fjb)
                    dk = []
                    for k in range(3):
                        dkt = scrp.tile([128, 128], bf16, tag=f"d{k}",
                                        name="dkt")
                        nc.vector.tensor_scalar_mul(
                            dkt[:], eye_sb[:],
                            E_sb[:, k * T + t: k * T + t + 1])
                        dk.append(dkt)
                    # whT chunks: psum[p, 128 q + n] = wh[node n, d = 128 q + p]
                    whp = psump.tile([128, D], f32, tag="wh")
                    for q in range(NQ):
                        for k in range(3):
                            nc.tensor.matmul(
                                whp[:, q * 128:(q + 1) * 128],
                                fb[k][:, q * 128:(q + 1) * 128], dk[k][:],
                                start=(k == 0), stop=(k == 2))
                    whs = whpool.tile([128, D], bf16, tag="whs")
                    nc.scalar.copy(whs[:], whp[:, 0:D])
                    # out = wh @ W : accumulate over d-chunks
                    outp = psump.tile([128, D], f32, tag="out")
                    for q in range(NQ):
                        nc.tensor.matmul(
                            outp[:, 0:512], whs[:, q * 128:(q + 1) * 128],
                            w_bf[:, q * D: q * D + 512],
                            start=(q == 0), stop=(q == NQ - 1))
                    for q in range(NQ):
                        nc.tensor.matmul(
                            outp[:, 512:D], whs[:, q * 128:(q + 1) * 128],
                            w_bf[:, q * D + 512:(q + 1) * D],
                            start=(q == 0), stop=(q == NQ - 1))
                    rel = whpool.tile([128, D], f32, tag="rel")
                    nc.scalar.activation(rel[:], outp[:, 0:D], Act.Relu)
                    res = whpool.tile([128, D], f32, tag="res")
                    nc.vector.tensor_add(res[:], rel[:], f0t[:])
                    nc.scalar.dma_start(out[t * 128:(t + 1) * 128, :],
                                        res[:])

    nc.compile()
    return nc


def make_in_maps(F_0, F_1, F_2, F_3, W, a, B=B_FULL, T=T_FULL, ncores=NCORES):
    BP = T * 128
    Fs = [np.asarray(x, np.float32) for x in (F_0, F_1, F_2, F_3)]
    d = Fs[0].shape[1]
    wf = np.ascontiguousarray(np.asarray(W, np.float32))
    a2 = np.ascontiguousarray(np.asarray(a, np.float32)[d:, 0]).reshape(1, d)
    eye = np.eye(128, dtype=np.float32)
    in_maps = []
    for c in range(ncores):
        m = {"w": wf, "a2row": a2, "eye": eye}
        for j in range(4):
            fj = np.zeros((BP, d), np.float32)
            fj[:B] = Fs[j][c * B:(c + 1) * B]
            m[f"f{j}"] = fj
        for r in range(3):
            q = 3 * c + r
            jj, b = q // ncores, q % ncores
            gr = np.zeros((BP, d), np.float32)
            gr[:B] = Fs[jj + 1][b * B:(b + 1) * B]
            m[f"g{r}"] = gr
        in_maps.append(m)
    return in_maps


_NC_CACHE = {}


def kernel(F_0, F_1, F_2, F_3, W, a):
    from concourse.bass_utils import run_bass_kernel_spmd
    if "nc" not in _NC_CACHE:
        _NC_CACHE["nc"] = build_kernel()
    nc = _NC_CACHE["nc"]
    in_maps = make_in_maps(F_0, F_1, F_2, F_3, W, a)
    res = run_bass_kernel_spmd(nc, in_maps, core_ids=list(range(NCORES)))
    out = np.concatenate(
        [res.results[c]["out"][:B_FULL] for c in range(NCORES)], axis=0)
    return np.ascontiguousarray(out, np.float32)


if __name__ == "__main__":
    nc = build_kernel()
    print("build ok")


# revision 8
# speedup vs baseline: 1.2107x; 1.0811x over previous
"""AdaptiveFusionLayer Trainium2 kernel — 8-core data-parallel, gather-free.

Math (derived from the reference):
  u2 = W @ a[D:]                               # [D]
  v  = concat([F_1 @ u2, F_2 @ u2, F_3 @ u2])  # [3N]  (block layout)
  e[n, :] = softmax_k(v[3n + k])               # Wh1 cancels (softmax shift-invariance)
  wh = e0*F_1 + e1*F_2 + e2*F_3                # row-wise
  out = relu(wh @ W) + F_0

Sharding: node-blocks of B = N/8 rows per core.  The score entries core c
needs, v[3Bc : 3B(c+1)), are exactly blocks q = 3c+r (r=0..2) of v, where
block q is F_{q//8 + 1}[ (q%8)·B : (q%8+1)·B ] @ u2.  The host hands core c
those three row-blocks as extra inputs g0..g2, so no collective is needed:
every core computes exactly the scores it consumes.
"""

import numpy as np

N, D, NCORES = 50000, 768, 8
B_FULL = N // NCORES           # 6250 real nodes per core
T_FULL = (B_FULL + 127) // 128  # 49 node-tiles (padded)


def build_kernel(B=B_FULL, T=T_FULL, stage=99):
    from concourse import bass, bacc, tile, mybir

    f32 = mybir.dt.float32
    bf16 = mybir.dt.bfloat16
    Alu = mybir.AluOpType
    Act = mybir.ActivationFunctionType

    BP = T * 128                # padded nodes per core
    VPAD = 3 * BP               # padded score vector length
    FULL_T = B // 128           # full 128-tiles of real nodes
    TAIL = B - FULL_T * 128     # real nodes in the last tile
    NQ = D // 128               # 6 chunks of the feature dim

    nc = bacc.Bacc(None, target_bir_lowering=False, debug=False)

    f = [nc.declare_dram_parameter(f"f{j}", [BP, D], f32, isOutput=False)
         for j in range(4)]
    g = [nc.declare_dram_parameter(f"g{r}", [BP, D], f32, isOutput=False)
         for r in range(3)]
    w = nc.declare_dram_parameter("w", [D, D], f32, isOutput=False)
    a2row = nc.declare_dram_parameter("a2row", [1, D], f32, isOutput=False)
    eye = nc.declare_dram_parameter("eye", [128, 128], f32, isOutput=False)
    out = nc.declare_dram_parameter("out", [BP, D], f32, isOutput=True)

    with tile.TileContext(nc) as tc:
        with (
            tc.tile_pool(name="const", bufs=1) as constp,
            tc.tile_pool(name="gpool", bufs=6) as gpool,
            tc.tile_pool(name="fpool", bufs=4) as fpool,
            tc.tile_pool(name="scr", bufs=2) as scrp,
            tc.tile_pool(name="whpool", bufs=3) as whpool,
            tc.tile_pool(name="psum", bufs=2, space="PSUM") as psump,
            tc.tile_pool(name="dram", bufs=1, space="DRAM") as dramp,
        ):
            def dummy_out():
                dummy = whpool.tile([128, D], f32, tag="res", name="dummy")
                nc.vector.memset(dummy[:], 0.0)
                for t in range(T):
                    nc.sync.dma_start(out[t * 128:(t + 1) * 128, :], dummy[:])

            # ---------------- setup ----------------
            w_sb = constp.tile([128, NQ * D], f32)
            for q in range(NQ):
                nc.sync.dma_start(w_sb[:, q * D:(q + 1) * D],
                                  w[q * 128:(q + 1) * 128, :])
            w_bf = constp.tile([128, NQ * D], bf16)
            for q in range(NQ):
                nc.scalar.copy(w_bf[:, q * D:(q + 1) * D],
                               w_sb[:, q * D:(q + 1) * D])
            a2_sb = constp.tile([1, D], f32)
            nc.sync.dma_start(a2_sb[:], a2row[:])
            eye_f32 = constp.tile([128, 128], f32)
            nc.sync.dma_start(eye_f32[:], eye[:])
            eye_sb = constp.tile([128, 128], bf16)
            nc.scalar.copy(eye_sb[:], eye_f32[:])
            ones_sb = constp.tile([1, 128], f32)
            nc.vector.memset(ones_sb[:], 1.0)

            # a2 broadcast across partitions via PE outer product
            bc_ps = psump.tile([128, D], f32, tag="wh")
            nc.tensor.matmul(bc_ps[:, 0:512], ones_sb[:], a2_sb[:, 0:512],
                             start=True, stop=True)
            nc.tensor.matmul(bc_ps[:, 512:D], ones_sb[:], a2_sb[:, 512:D],
                             start=True, stop=True)
            a2b_sb = constp.tile([128, D], f32)
            nc.scalar.copy(a2b_sb[:], bc_ps[:, 0:D])

            # u2 = W @ a2 : chunk q on partitions -> u2col[:, q]
            u2col = constp.tile([128, NQ], f32)
            for q in range(NQ):
                ttr_scr = scrp.tile([128, D], f32, tag="ttr", name="ttr_scr")
                nc.vector.affine_mul_reduce(
                    out=ttr_scr[:], accum_out=u2col[:, q:q + 1],
                    in0=w_sb[:, q * D:(q + 1) * D], in1=a2b_sb[:],
                    scale=1.0, bias=0.0)
            # u2col [128, NQ] -> u2row [1, D]  (d = 128 q + p)
            u2row = constp.tile([1, D], f32)
            for q in range(NQ):
                nc.sync.dma_start(
                    u2row[:, q * 128:(q + 1) * 128]
                    .rearrange("o (p z) -> o p z", z=1),
                    u2col[:, q:q + 1])
            # u2 broadcast across partitions
            u2b_ps = psump.tile([128, D], f32, tag="wh")
            nc.tensor.matmul(u2b_ps[:, 0:512], ones_sb[:], u2row[:, 0:512],
                             start=True, stop=True)
            nc.tensor.matmul(u2b_ps[:, 512:D], ones_sb[:], u2row[:, 512:D],
                             start=True, stop=True)
            u2b_sb = constp.tile([128, D], f32)
            nc.scalar.copy(u2b_sb[:], u2b_ps[:, 0:D])

            done = stage < 2
            if done:
                dummy_out()

            # ---------------- phase A: scores ----------------
            if not done:
                sig_sb = constp.tile([128, 3 * T], f32)   # col r*T + t
                for r in range(3):
                    for t in range(T):
                        gt = gpool.tile([128, D], f32, tag="g", name="gt")
                        nc.sync.dma_start(gt[:], g[r][t * 128:(t + 1) * 128, :])
                        ttr_scr = scrp.tile([128, D], f32, tag="ttr",
                                            name="ttr_scr")
                        nc.vector.affine_mul_reduce(
                            out=ttr_scr[:],
                            accum_out=sig_sb[:, r * T + t: r * T + t + 1],
                            in0=gt[:], in1=u2b_sb[:], scale=1.0, bias=0.0)

                # transpose scores to [t, p] and scatter contiguously:
                # vloc[r*B + 128 t + p]
                vloc = dramp.tile([VPAD], f32)
                for r in range(3):
                    sigT_ps = psump.tile([T, 128], f32, tag="wh",
                                         name="sigT_ps")
                    nc.tensor.transpose(sigT_ps[:],
                                        sig_sb[:, r * T:(r + 1) * T],
                                        eye_f32[:])
                    sigT_sb = scrp.tile([T, 128], f32, tag="sigT",
                                        name="sigT_sb")
                    nc.scalar.copy(sigT_sb[:], sigT_ps[:])
                    if FULL_T:
                        nc.sync.dma_start(
                            vloc[r * B: r * B + FULL_T * 128]
                            .rearrange("(t p) -> t p", p=128),
                            sigT_sb[0:FULL_T, :])
                    if TAIL:
                        nc.sync.dma_start(
                            vloc[r * B + FULL_T * 128: (r + 1) * B]
                            .rearrange("(o p) -> o p", o=1),
                            sigT_sb[FULL_T:FULL_T + 1, 0:TAIL])
                # zero the padded tail so exp() stays finite
                if VPAD > 3 * B:
                    zpad = constp.tile([VPAD - 3 * B, 1], f32)
                    nc.vector.memset(zpad[:], 0.0)
                    nc.sync.dma_start(
                        vloc[3 * B: VPAD].rearrange("(p o) -> p o", o=1),
                        zpad[:])
                if stage < 3:
                    dummy_out()
                    done = True

            # ---------------- softmax ----------------
            if not done:
                # V2[t, c] = vloc[384 t + c]; L_k[p, t] = V2[t, 3p + k]
                V2_sb = constp.tile([T, 3 * 128], f32)
                nc.sync.dma_start(
                    V2_sb[:], vloc[:].rearrange("(t c) -> t c", c=384))
                Ps = []
                for k in range(3):
                    Lk_ps = psump.tile([128, T], f32, tag="wh", name="Lk_ps")
                    nc.tensor.transpose(
                        Lk_ps[:],
                        V2_sb.rearrange("t (p k) -> t p k", k=3)[:, :, k],
                        eye_f32[0:T, 0:T])
                    Pk = constp.tile([128, T], f32, name=f"P{k}_sb",
                                     tag=f"P{k}")
                    nc.scalar.activation(Pk[:], Lk_ps[:], Act.Exp)
                    Ps.append(Pk)
                S_sb = constp.tile([128, T], f32)
                nc.vector.tensor_add(S_sb[:], Ps[0][:], Ps[1][:])
                S2_sb = constp.tile([128, T], f32)
                nc.vector.tensor_add(S2_sb[:], S_sb[:], Ps[2][:])
                R_sb = constp.tile([128, T], f32)
                nc.vector.reciprocal(R_sb[:], S2_sb[:])
                E_sb = constp.tile([128, 3 * T], f32)  # col k*T + tau
                for k in range(3):
                    nc.vector.tensor_mul(
                        E_sb[:, k * T:(k + 1) * T], Ps[k][:], R_sb[:])
                if stage < 4:
                    dummy_out()
                    done = True

            # ---------------- phase B ----------------
            if not done:
                for t in range(T):
                    f0t = fpool.tile([128, D], f32, tag="f0", name="f0t")
                    nc.gpsimd.dma_start(f0t[:], f[0][t * 128:(t + 1) * 128, :])
                    fb = []
                    for j in (1, 2, 3):
                        fjt = fpool.tile([128, D], f32, tag=f"f{j}",
                                         name="fjt")
                        eng = nc.scalar if j == 2 else nc.sync
                        eng.dma_start(fjt[:],
                                      f[j][t * 128:(t + 1) * 128, :])
                        # e_k-weighted bf16 cast: fjb = e_{j-1}[node] * f_j
                        fjb = fpool.tile([128, D], bf16, tag=f"f{j}b",
                                         name="fjb")
                        nc.scalar.activation(
                            fjb[:], fjt[:], Act.Copy,
                            scale=E_sb[:, (j - 1) * T + t: (j - 1) * T + t + 1])
                        fb.append(fjb)
                    # wh = fb0 + fb1 + fb2 (bf16, node-major)
                    wha = whpool.tile([128, D], bf16, tag="wha")
                    nc.vector.tensor_add(wha[:], fb[0][:], fb[1][:])
                    whb = whpool.tile([128, D], bf16, tag="whb")
                    nc.vector.tensor_add(whb[:], wha[:], fb[2][:])
                    # transpose to whT chunks: psum[p, 128q + n] = wh[n, 128q+p]
                    whp = psump.tile([128, D], bf16, tag="wh")
                    for q in range(NQ):
                        nc.tensor.transpose(
                            whp[:, q * 128:(q + 1) * 128],
                            whb[:, q * 128:(q + 1) * 128], eye_sb[:])
                    whs = whpool.tile([128, D], bf16, tag="whs")
                    nc.scalar.copy(whs[:], whp[:, 0:D])
                    # out = wh @ W : accumulate over d-chunks
                    outp = psump.tile([128, D], f32, tag="out")
                    for q in range(NQ):
                        nc.tensor.matmul(
                            outp[:, 0:512], whs[:, q * 128:(q + 1) * 128],
                            w_bf[:, q * D: q * D + 512],
                            start=(q == 0), stop=(q == NQ - 1))
                    for q in range(NQ):
                        nc.tensor.matmul(
                            outp[:, 512:D], whs[:, q * 128:(q + 1) * 128],
                            w_bf[:, q * D + 512:(q + 1) * D],
                            start=(q == 0), stop=(q == NQ - 1))
                    rel = whpool.tile([128, D], f32, tag="rel")
                    nc.scalar.activation(rel[:], outp[:, 0:D], Act.Relu)
                    res = whpool.tile([128, D], f32, tag="res")
                    nc.vector.tensor_add(res[:], rel[:], f0t[:])
                    nc.scalar.dma_start(out[t * 128:(t + 1) * 128, :],
                                        res[:])

    nc.compile()
    return nc


def make_in_maps(F_0, F_1, F_2, F_3, W, a, B=B_FULL, T=T_FULL, ncores=NCORES):
    BP = T * 128
    Fs = [np.asarray(x, np.float32) for x in (F_0, F_1, F_2, F_3)]
    d = Fs[0].shape[1]
    wf = np.ascontiguousarray(np.asarray(W, np.float32))
    a2 = np.ascontiguousarray(np.asarray(a, np.float32)[d:, 0]).reshape(1, d)
    eye = np.eye(128, dtype=np.float32)
    in_maps = []
    for c in range(ncores):
        m = {"w": wf, "a2row": a2, "eye": eye}
        for j in range(4):
            fj = np.zeros((BP, d), np.float32)
            fj[:B] = Fs[j][c * B:(c + 1) * B]
            m[f"f{j}"] = fj
        for r in range(3):
            q = 3 * c + r
            jj, b = q // ncores, q % ncores
            gr = np.zeros((BP, d), np.float32)
            gr[:B] = Fs[jj + 1][b * B:(b + 1) * B]
            m[f"g{r}"] = gr
        in_maps.append(m)
    return in_maps


_NC_CACHE = {}


def kernel(F_0, F_1, F_2, F_3, W, a):
    from concourse.bass_utils import run_bass_kernel_spmd
    if "nc" not in _NC_CACHE:
        _NC_CACHE["nc"] = build_kernel()
    nc = _NC_CACHE["nc"]
    in_maps = make_in_maps(F_0, F_1, F_2, F_3, W, a)
    res = run_bass_kernel_spmd(nc, in_maps, core_ids=list(range(NCORES)))
    out = np.concatenate(
        [res.results[c]["out"][:B_FULL] for c in range(NCORES)], axis=0)
    return np.ascontiguousarray(out, np.float32)


if __name__ == "__main__":
    nc = build_kernel()
    print("build ok")


# revision 9
# speedup vs baseline: 1.2213x; 1.0088x over previous
"""AdaptiveFusionLayer Trainium2 kernel — 8-core data-parallel, gather-free.

Math (derived from the reference):
  u2 = W @ a[D:]                               # [D]
  v  = concat([F_1 @ u2, F_2 @ u2, F_3 @ u2])  # [3N]  (block layout)
  e[n, :] = softmax_k(v[3n + k])               # Wh1 cancels (softmax shift-invariance)
  wh = e0*F_1 + e1*F_2 + e2*F_3                # row-wise
  out = relu(wh @ W) + F_0

Sharding: node-blocks of B = N/8 rows per core.  The score entries core c
needs, v[3Bc : 3B(c+1)), are exactly blocks q = 3c+r (r=0..2) of v, where
block q is F_{q//8 + 1}[ (q%8)·B : (q%8+1)·B ] @ u2.  The host hands core c
those three row-blocks as extra inputs g0..g2, so no collective is needed:
every core computes exactly the scores it consumes.
"""

import numpy as np

N, D, NCORES = 50000, 768, 8
B_FULL = N // NCORES           # 6250 real nodes per core
T_FULL = (B_FULL + 127) // 128  # 49 node-tiles (padded)


def build_kernel(B=B_FULL, T=T_FULL, stage=99):
    from concourse import bass, bacc, tile, mybir

    f32 = mybir.dt.float32
    bf16 = mybir.dt.bfloat16
    Alu = mybir.AluOpType
    Act = mybir.ActivationFunctionType

    BP = T * 128                # padded nodes per core
    VPAD = 3 * BP               # padded score vector length
    FULL_T = B // 128           # full 128-tiles of real nodes
    TAIL = B - FULL_T * 128     # real nodes in the last tile
    NQ = D // 128               # 6 chunks of the feature dim

    nc = bacc.Bacc(None, target_bir_lowering=False, debug=False)

    f = [nc.declare_dram_parameter(f"f{j}", [BP, D], f32, isOutput=False)
         for j in range(4)]
    g = [nc.declare_dram_parameter(f"g{r}", [BP, D], f32, isOutput=False)
         for r in range(3)]
    w = nc.declare_dram_parameter("w", [D, D], f32, isOutput=False)
    a2row = nc.declare_dram_parameter("a2row", [1, D], f32, isOutput=False)
    eye = nc.declare_dram_parameter("eye", [128, 128], f32, isOutput=False)
    out = nc.declare_dram_parameter("out", [BP, D], f32, isOutput=True)

    with tile.TileContext(nc) as tc:
        with (
            tc.tile_pool(name="const", bufs=1) as constp,
            tc.tile_pool(name="gpool", bufs=6) as gpool,
            tc.tile_pool(name="fpool", bufs=4) as fpool,
            tc.tile_pool(name="scr", bufs=2) as scrp,
            tc.tile_pool(name="whpool", bufs=3) as whpool,
            tc.tile_pool(name="psum", bufs=2, space="PSUM") as psump,
            tc.tile_pool(name="dram", bufs=1, space="DRAM") as dramp,
        ):
            def dummy_out():
                dummy = whpool.tile([128, D], f32, tag="res", name="dummy")
                nc.vector.memset(dummy[:], 0.0)
                for t in range(T):
                    nc.sync.dma_start(out[t * 128:(t + 1) * 128, :], dummy[:])

            # ---------------- setup ----------------
            w_sb = constp.tile([128, NQ * D], f32)
            for q in range(NQ):
                nc.sync.dma_start(w_sb[:, q * D:(q + 1) * D],
                                  w[q * 128:(q + 1) * 128, :])
            w_bf = constp.tile([128, NQ * D], bf16)
            for q in range(NQ):
                nc.scalar.copy(w_bf[:, q * D:(q + 1) * D],
                               w_sb[:, q * D:(q + 1) * D])
            a2_sb = constp.tile([1, D], f32)
            nc.sync.dma_start(a2_sb[:], a2row[:])
            eye_f32 = constp.tile([128, 128], f32)
            nc.sync.dma_start(eye_f32[:], eye[:])
            eye_sb = constp.tile([128, 128], bf16)
            nc.scalar.copy(eye_sb[:], eye_f32[:])
            ones_sb = constp.tile([1, 128], f32)
            nc.vector.memset(ones_sb[:], 1.0)

            # a2 broadcast across partitions via PE outer product
            bc_ps = psump.tile([128, D], f32, tag="out")
            nc.tensor.matmul(bc_ps[:, 0:512], ones_sb[:], a2_sb[:, 0:512],
                             start=True, stop=True)
            nc.tensor.matmul(bc_ps[:, 512:D], ones_sb[:], a2_sb[:, 512:D],
                             start=True, stop=True)
            a2b_sb = constp.tile([128, D], f32)
            nc.scalar.copy(a2b_sb[:], bc_ps[:, 0:D])

            # u2 = W @ a2 : chunk q on partitions -> u2col[:, q]
            u2col = constp.tile([128, NQ], f32)
            for q in range(NQ):
                ttr_scr = scrp.tile([128, D], f32, tag="ttr", name="ttr_scr")
                nc.vector.affine_mul_reduce(
                    out=ttr_scr[:], accum_out=u2col[:, q:q + 1],
                    in0=w_sb[:, q * D:(q + 1) * D], in1=a2b_sb[:],
                    scale=1.0, bias=0.0)
            # u2col [128, NQ] -> u2row [1, D]  (d = 128 q + p)
            u2row = constp.tile([1, D], f32)
            for q in range(NQ):
                nc.sync.dma_start(
                    u2row[:, q * 128:(q + 1) * 128]
                    .rearrange("o (p z) -> o p z", z=1),
                    u2col[:, q:q + 1])
            # u2 broadcast across partitions
            u2b_ps = psump.tile([128, D], f32, tag="out")
            nc.tensor.matmul(u2b_ps[:, 0:512], ones_sb[:], u2row[:, 0:512],
                             start=True, stop=True)
            nc.tensor.matmul(u2b_ps[:, 512:D], ones_sb[:], u2row[:, 512:D],
                             start=True, stop=True)
            u2b_sb = constp.tile([128, D], f32)
            nc.scalar.copy(u2b_sb[:], u2b_ps[:, 0:D])

            done = stage < 2
            if done:
                dummy_out()

            # ---------------- phase A: scores ----------------
            if not done:
                sig_sb = constp.tile([128, 3 * T], f32)   # col r*T + t
                for r in range(3):
                    for t in range(T):
                        gt = gpool.tile([128, D], f32, tag="g", name="gt")
                        nc.sync.dma_start(gt[:], g[r][t * 128:(t + 1) * 128, :])
                        ttr_scr = scrp.tile([128, D], f32, tag="ttr",
                                            name="ttr_scr")
                        nc.vector.affine_mul_reduce(
                            out=ttr_scr[:],
                            accum_out=sig_sb[:, r * T + t: r * T + t + 1],
                            in0=gt[:], in1=u2b_sb[:], scale=1.0, bias=0.0)

                # transpose scores to [t, p] and scatter contiguously:
                # vloc[r*B + 128 t + p]
                vloc = dramp.tile([VPAD], f32)
                for r in range(3):
                    sigT_ps = psump.tile([T, 128], f32, tag="out",
                                         name="sigT_ps")
                    nc.tensor.transpose(sigT_ps[:],
                                        sig_sb[:, r * T:(r + 1) * T],
                                        eye_f32[:])
                    sigT_sb = scrp.tile([T, 128], f32, tag="sigT",
                                        name="sigT_sb")
                    nc.scalar.copy(sigT_sb[:], sigT_ps[:])
                    if FULL_T:
                        nc.sync.dma_start(
                            vloc[r * B: r * B + FULL_T * 128]
                            .rearrange("(t p) -> t p", p=128),
                            sigT_sb[0:FULL_T, :])
                    if TAIL:
                        nc.sync.dma_start(
                            vloc[r * B + FULL_T * 128: (r + 1) * B]
                            .rearrange("(o p) -> o p", o=1),
                            sigT_sb[FULL_T:FULL_T + 1, 0:TAIL])
                # zero the padded tail so exp() stays finite
                if VPAD > 3 * B:
                    zpad = constp.tile([VPAD - 3 * B, 1], f32)
                    nc.vector.memset(zpad[:], 0.0)
                    nc.sync.dma_start(
                        vloc[3 * B: VPAD].rearrange("(p o) -> p o", o=1),
                        zpad[:])
                if stage < 3:
                    dummy_out()
                    done = True

            # ---------------- softmax ----------------
            if not done:
                # V2[t, c] = vloc[384 t + c]; L_k[p, t] = V2[t, 3p + k]
                V2_sb = constp.tile([T, 3 * 128], f32)
                nc.sync.dma_start(
                    V2_sb[:], vloc[:].rearrange("(t c) -> t c", c=384))
                Ps = []
                for k in range(3):
                    Lk_ps = psump.tile([128, T], f32, tag="out", name="Lk_ps")
                    nc.tensor.transpose(
                        Lk_ps[:],
                        V2_sb.rearrange("t (p k) -> t p k", k=3)[:, :, k],
                        eye_f32[0:T, 0:T])
                    Pk = constp.tile([128, T], f32, name=f"P{k}_sb",
                                     tag=f"P{k}")
                    nc.scalar.activation(Pk[:], Lk_ps[:], Act.Exp)
                    Ps.append(Pk)
                S_sb = constp.tile([128, T], f32)
                nc.vector.tensor_add(S_sb[:], Ps[0][:], Ps[1][:])
                S2_sb = constp.tile([128, T], f32)
                nc.vector.tensor_add(S2_sb[:], S_sb[:], Ps[2][:])
                R_sb = constp.tile([128, T], f32)
                nc.vector.reciprocal(R_sb[:], S2_sb[:])
                E_sb = constp.tile([128, 3 * T], f32)  # col k*T + tau
                for k in range(3):
                    nc.vector.tensor_mul(
                        E_sb[:, k * T:(k + 1) * T], Ps[k][:], R_sb[:])
                if stage < 4:
                    dummy_out()
                    done = True

            # ---------------- phase B ----------------
            if not done:
                for t in range(T):
                    f0t = fpool.tile([128, D], f32, tag="f0", name="f0t")
                    nc.gpsimd.dma_start(f0t[:], f[0][t * 128:(t + 1) * 128, :])
                    fb = []
                    for j in (1, 2, 3):
                        fjt = fpool.tile([128, D], f32, tag=f"f{j}",
                                         name="fjt")
                        eng = nc.scalar if j == 2 else nc.sync
                        eng.dma_start(fjt[:],
                                      f[j][t * 128:(t + 1) * 128, :])
                        # e_k-weighted bf16 cast: fjb = e_{j-1}[node] * f_j
                        fjb = fpool.tile([128, D], bf16, tag=f"f{j}b",
                                         name="fjb")
                        nc.scalar.activation(
                            fjb[:], fjt[:], Act.Copy,
                            scale=E_sb[:, (j - 1) * T + t: (j - 1) * T + t + 1])
                        fb.append(fjb)
                    # wh = fb0 + fb1 + fb2 (bf16, node-major)
                    wha = whpool.tile([128, D], bf16, tag="wha")
                    nc.vector.tensor_add(wha[:], fb[0][:], fb[1][:])
                    whb = whpool.tile([128, D], bf16, tag="whb")
                    nc.vector.tensor_add(whb[:], wha[:], fb[2][:])
                    # transpose to whT chunks: psum[p, 128q + n] = wh[n, 128q+p]
                    whp = psump.tile([128, D], bf16, tag="wh", bufs=4)
                    for q in range(NQ):
                        nc.tensor.transpose(
                            whp[:, q * 128:(q + 1) * 128],
                            whb[:, q * 128:(q + 1) * 128], eye_sb[:])
                    whs = whpool.tile([128, D], bf16, tag="whs")
                    nc.scalar.copy(whs[:], whp[:, 0:D])
                    # out = wh @ W : accumulate over d-chunks
                    outp = psump.tile([128, D], f32, tag="out")
                    for q in range(NQ):
                        nc.tensor.matmul(
                            outp[:, 0:512], whs[:, q * 128:(q + 1) * 128],
                            w_bf[:, q * D: q * D + 512],
                            start=(q == 0), stop=(q == NQ - 1))
                    for q in range(NQ):
                        nc.tensor.matmul(
                            outp[:, 512:D], whs[:, q * 128:(q + 1) * 128],
                            w_bf[:, q * D + 512:(q + 1) * D],
                            start=(q == 0), stop=(q == NQ - 1))
                    rel = whpool.tile([128, D], f32, tag="rel")
                    nc.scalar.activation(rel[:], outp[:, 0:D], Act.Relu)
                    res = whpool.tile([128, D], f32, tag="res")
                    nc.vector.tensor_add(res[:], rel[:], f0t[:])
                    nc.scalar.dma_start(out[t * 128:(t + 1) * 128, :],
                                        res[:])

    nc.compile()
    return nc


def make_in_maps(F_0, F_1, F_2, F_3, W, a, B=B_FULL, T=T_FULL, ncores=NCORES):
    BP = T * 128
    Fs = [np.asarray(x, np.float32) for x in (F_0, F_1, F_2, F_3)]
    d = Fs[0].shape[1]
    wf = np.ascontiguousarray(np.asarray(W, np.float32))
    a2 = np.ascontiguousarray(np.asarray(a, np.float32)[d:, 0]).reshape(1, d)
    eye = np.eye(128, dtype=np.float32)
    in_maps = []
    for c in range(ncores):
        m = {"w": wf, "a2row": a2, "eye": eye}
        for j in range(4):
            fj = np.zeros((BP, d), np.float32)
            fj[:B] = Fs[j][c * B:(c + 1) * B]
            m[f"f{j}"] = fj
        for r in range(3):
            q = 3 * c + r
            jj, b = q // ncores, q % ncores
            gr = np.zeros((BP, d), np.float32)
            gr[:B] = Fs[jj + 1][b * B:(b + 1) * B]
            m[f"g{r}"] = gr
        in_maps.append(m)
    return in_maps


_NC_CACHE = {}


def kernel(F_0, F_1, F_2, F_3, W, a):
    from concourse.bass_utils import run_bass_kernel_spmd
    if "nc" not in _NC_CACHE:
        _NC_CACHE["nc"] = build_kernel()
    nc = _NC_CACHE["nc"]
    in_maps = make_in_maps(F_0, F_1, F_2, F_3, W, a)
    res = run_bass_kernel_spmd(nc, in_maps, core_ids=list(range(NCORES)))
    out = np.concatenate(
        [res.results[c]["out"][:B_FULL] for c in range(NCORES)], axis=0)
    return np.ascontiguousarray(out, np.float32)


if __name__ == "__main__":
    nc = build_kernel()
    print("build ok")


# revision 10
# speedup vs baseline: 1.4424x; 1.1810x over previous
"""AdaptiveFusionLayer Trainium2 kernel — 8-core data-parallel, gather-free.

Math (derived from the reference):
  u2 = W @ a[D:]                               # [D]
  v  = concat([F_1 @ u2, F_2 @ u2, F_3 @ u2])  # [3N]  (block layout)
  e[n, :] = softmax_k(v[3n + k])               # Wh1 cancels (softmax shift-invariance)
  wh = e0*F_1 + e1*F_2 + e2*F_3                # row-wise
  out = relu(wh @ W) + F_0

Sharding: node-blocks of B = N/8 rows per core.  The score entries core c
needs, v[3Bc : 3B(c+1)), are exactly blocks q = 3c+r (r=0..2) of v, where
block q is F_{q//8 + 1}[ (q%8)·B : (q%8+1)·B ] @ u2.  The host hands core c
those three row-blocks as extra inputs g0..g2, so no collective is needed:
every core computes exactly the scores it consumes.
"""

import numpy as np

N, D, NCORES = 50000, 768, 8
B_FULL = N // NCORES           # 6250 real nodes per core
T_FULL = (B_FULL + 127) // 128  # 49 node-tiles (padded)


def build_kernel(B=B_FULL, T=T_FULL, stage=99):
    from concourse import bass, bacc, tile, mybir

    f32 = mybir.dt.float32
    bf16 = mybir.dt.bfloat16
    Alu = mybir.AluOpType
    Act = mybir.ActivationFunctionType

    BP = T * 128                # padded nodes per core
    VPAD = 3 * BP               # padded score vector length
    FULL_T = B // 128           # full 128-tiles of real nodes
    TAIL = B - FULL_T * 128     # real nodes in the last tile
    NQ = D // 128               # 6 chunks of the feature dim

    nc = bacc.Bacc(None, target_bir_lowering=False, debug=False)

    f = [nc.declare_dram_parameter(f"f{j}", [BP, D], f32, isOutput=False)
         for j in range(4)]
    g = [nc.declare_dram_parameter(f"g{r}", [BP, D], f32, isOutput=False)
         for r in range(3)]
    w = nc.declare_dram_parameter("w", [D, D], f32, isOutput=False)
    a2row = nc.declare_dram_parameter("a2row", [1, D], f32, isOutput=False)
    eye = nc.declare_dram_parameter("eye", [128, 128], f32, isOutput=False)
    out = nc.declare_dram_parameter("out", [BP, D], f32, isOutput=True)

    with tile.TileContext(nc) as tc:
        with (
            tc.tile_pool(name="const", bufs=1) as constp,
            tc.tile_pool(name="gpool", bufs=3) as gpool,
            tc.tile_pool(name="fpool", bufs=4) as fpool,
            tc.tile_pool(name="scr", bufs=2) as scrp,
            tc.tile_pool(name="whpool", bufs=3) as whpool,
            tc.tile_pool(name="psum", bufs=2, space="PSUM") as psump,
            tc.tile_pool(name="dram", bufs=1, space="DRAM") as dramp,
        ):
            def dummy_out():
                dummy = whpool.tile([128, D], f32, tag="res", name="dummy")
                nc.vector.memset(dummy[:], 0.0)
                for t in range(T):
                    nc.sync.dma_start(out[t * 128:(t + 1) * 128, :], dummy[:])

            # ---------------- setup ----------------
            w_sb = constp.tile([128, NQ * D], f32)
            for q in range(NQ):
                nc.sync.dma_start(w_sb[:, q * D:(q + 1) * D],
                                  w[q * 128:(q + 1) * 128, :])
            w_bf = constp.tile([128, NQ * D], bf16)
            for q in range(NQ):
                nc.scalar.copy(w_bf[:, q * D:(q + 1) * D],
                               w_sb[:, q * D:(q + 1) * D])
            a2_sb = constp.tile([1, D], f32)
            nc.sync.dma_start(a2_sb[:], a2row[:])
            eye_f32 = constp.tile([128, 128], f32)
            nc.sync.dma_start(eye_f32[:], eye[:])
            eye_sb = constp.tile([128, 128], bf16)
            nc.scalar.copy(eye_sb[:], eye_f32[:])
            ones_sb = constp.tile([1, 128], f32)
            nc.vector.memset(ones_sb[:], 1.0)

            # a2 broadcast across partitions via PE outer product
            bc_ps = psump.tile([128, D], f32, tag="out")
            nc.tensor.matmul(bc_ps[:, 0:512], ones_sb[:], a2_sb[:, 0:512],
                             start=True, stop=True)
            nc.tensor.matmul(bc_ps[:, 512:D], ones_sb[:], a2_sb[:, 512:D],
                             start=True, stop=True)
            a2b_sb = constp.tile([128, D], f32)
            nc.scalar.copy(a2b_sb[:], bc_ps[:, 0:D])

            # u2 = W @ a2 : chunk q on partitions -> u2col[:, q]
            u2col = constp.tile([128, NQ], f32)
            for q in range(NQ):
                ttr_scr = scrp.tile([128, D], f32, tag="ttr", name="ttr_scr")
                nc.vector.affine_mul_reduce(
                    out=ttr_scr[:], accum_out=u2col[:, q:q + 1],
                    in0=w_sb[:, q * D:(q + 1) * D], in1=a2b_sb[:],
                    scale=1.0, bias=0.0)
            # u2col [128, NQ] -> u2row [1, D]  (d = 128 q + p)
            u2row = constp.tile([1, D], f32)
            for q in range(NQ):
                nc.sync.dma_start(
                    u2row[:, q * 128:(q + 1) * 128]
                    .rearrange("o (p z) -> o p z", z=1),
                    u2col[:, q:q + 1])
            # u2 broadcast across partitions
            u2b_ps = psump.tile([128, D], f32, tag="out")
            nc.tensor.matmul(u2b_ps[:, 0:512], ones_sb[:], u2row[:, 0:512],
                             start=True, stop=True)
            nc.tensor.matmul(u2b_ps[:, 512:D], ones_sb[:], u2row[:, 512:D],
                             start=True, stop=True)
            u2b_sb = constp.tile([128, D], f32)
            nc.scalar.copy(u2b_sb[:], u2b_ps[:, 0:D])

            done = stage < 2
            if done:
                dummy_out()

            # ---------------- phase A: scores ----------------
            if not done:
                sig_sb = constp.tile([128, 3 * T], f32)   # col r*T + t
                for r in range(3):
                    for t3 in range(0, T, 3):
                        nb = min(3, T - t3)
                        gt = gpool.tile([128, 3 * D], f32, tag="g", name="gt")
                        nc.sync.dma_start(
                            gt[:, 0:nb * D].rearrange("p (a d) -> p a d", d=D),
                            g[r][t3 * 128:(t3 + nb) * 128, :]
                            .rearrange("(a p) d -> p a d", p=128))
                        for a in range(nb):
                            t = t3 + a
                            ttr_scr = scrp.tile([128, D], f32, tag="ttr",
                                                name="ttr_scr")
                            nc.vector.affine_mul_reduce(
                                out=ttr_scr[:],
                                accum_out=sig_sb[:, r * T + t: r * T + t + 1],
                                in0=gt[:, a * D:(a + 1) * D], in1=u2b_sb[:],
                                scale=1.0, bias=0.0)

                # transpose scores to [t, p] and scatter contiguously:
                # vloc[r*B + 128 t + p]
                vloc = dramp.tile([VPAD], f32)
                for r in range(3):
                    sigT_ps = psump.tile([T, 128], f32, tag="out",
                                         name="sigT_ps")
                    nc.tensor.transpose(sigT_ps[:],
                                        sig_sb[:, r * T:(r + 1) * T],
                                        eye_f32[:])
                    sigT_sb = scrp.tile([T, 128], f32, tag="sigT",
                                        name="sigT_sb")
                    nc.scalar.copy(sigT_sb[:], sigT_ps[:])
                    if FULL_T:
                        nc.sync.dma_start(
                            vloc[r * B: r * B + FULL_T * 128]
                            .rearrange("(t p) -> t p", p=128),
                            sigT_sb[0:FULL_T, :])
                    if TAIL:
                        nc.sync.dma_start(
                            vloc[r * B + FULL_T * 128: (r + 1) * B]
                            .rearrange("(o p) -> o p", o=1),
                            sigT_sb[FULL_T:FULL_T + 1, 0:TAIL])
                # zero the padded tail so exp() stays finite
                if VPAD > 3 * B:
                    zpad = constp.tile([VPAD - 3 * B, 1], f32)
                    nc.vector.memset(zpad[:], 0.0)
                    nc.sync.dma_start(
                        vloc[3 * B: VPAD].rearrange("(p o) -> p o", o=1),
                        zpad[:])
                if stage < 3:
                    dummy_out()
                    done = True

            # ---------------- softmax ----------------
            if not done:
                # V2[t, c] = vloc[384 t + c]; L_k[p, t] = V2[t, 3p + k]
                V2_sb = constp.tile([T, 3 * 128], f32)
                nc.sync.dma_start(
                    V2_sb[:], vloc[:].rearrange("(t c) -> t c", c=384))
                Ps = []
                for k in range(3):
                    Lk_ps = psump.tile([128, T], f32, tag="out", name="Lk_ps")
                    nc.tensor.transpose(
                        Lk_ps[:],
                        V2_sb.rearrange("t (p k) -> t p k", k=3)[:, :, k],
                        eye_f32[0:T, 0:T])
                    Pk = constp.tile([128, T], f32, name=f"P{k}_sb",
                                     tag=f"P{k}")
                    nc.scalar.activation(Pk[:], Lk_ps[:], Act.Exp)
                    Ps.append(Pk)
                S_sb = constp.tile([128, T], f32)
                nc.vector.tensor_add(S_sb[:], Ps[0][:], Ps[1][:])
                S2_sb = constp.tile([128, T], f32)
                nc.vector.tensor_add(S2_sb[:], S_sb[:], Ps[2][:])
                R_sb = constp.tile([128, T], f32)
                nc.vector.reciprocal(R_sb[:], S2_sb[:])
                E_sb = constp.tile([128, 3 * T], f32)  # col k*T + tau
                for k in range(3):
                    nc.vector.tensor_mul(
                        E_sb[:, k * T:(k + 1) * T], Ps[k][:], R_sb[:])
                if stage < 4:
                    dummy_out()
                    done = True

            # ---------------- phase B ----------------
            if not done:
                def front_end(t):
                    f0t = fpool.tile([128, D], f32, tag="f0", name="f0t")
                    nc.gpsimd.dma_start(f0t[:], f[0][t * 128:(t + 1) * 128, :])
                    fb = []
                    for j in (1, 2, 3):
                        fjt = fpool.tile([128, D], f32, tag=f"f{j}",
                                         name="fjt")
                        nc.sync.dma_start(fjt[:],
                                          f[j][t * 128:(t + 1) * 128, :])
                        # e_k-weighted bf16 cast on DVE (2x mode)
                        fjb = fpool.tile([128, D], bf16, tag=f"f{j}b",
                                         name="fjb")
                        nc.vector.tensor_scalar_mul(
                            fjb[:], fjt[:],
                            E_sb[:, (j - 1) * T + t: (j - 1) * T + t + 1])
                        fb.append(fjb)
                    # wh = fb0 + fb1 + fb2 (bf16, node-major)
                    wha = whpool.tile([128, D], bf16, tag="wha")
                    nc.vector.tensor_add(wha[:], fb[0][:], fb[1][:])
                    whb = whpool.tile([128, D], bf16, tag="whb")
                    nc.vector.tensor_add(whb[:], wha[:], fb[2][:])
                    # transpose: whp[p, 128q + n] = wh[n, 128q + p]
                    whp = psump.tile([128, D], bf16, tag="wh", bufs=4)
                    for q in range(NQ):
                        nc.tensor.transpose(
                            whp[:, q * 128:(q + 1) * 128],
                            whb[:, q * 128:(q + 1) * 128], eye_sb[:])
                    whs = whpool.tile([128, D], bf16, tag="whs")
                    nc.scalar.copy(whs[:], whp[:, 0:D])
                    return f0t, whs

                def back_end(t, f0t, whs):
                    outp = psump.tile([128, D], f32, tag="out")
                    for q in range(NQ):
                        nc.tensor.matmul(
                            outp[:, 0:512], whs[:, q * 128:(q + 1) * 128],
                            w_bf[:, q * D: q * D + 512],
                            start=(q == 0), stop=(q == NQ - 1))
                    for q in range(NQ):
                        nc.tensor.matmul(
                            outp[:, 512:D], whs[:, q * 128:(q + 1) * 128],
                            w_bf[:, q * D + 512:(q + 1) * D],
                            start=(q == 0), stop=(q == NQ - 1))
                    rel = whpool.tile([128, D], f32, tag="rel")
                    nc.scalar.activation(rel[:], outp[:, 0:D], Act.Relu)
                    res = whpool.tile([128, D], f32, tag="res")
                    nc.vector.tensor_add(res[:], rel[:], f0t[:])
                    nc.scalar.dma_start(out[t * 128:(t + 1) * 128, :],
                                        res[:])

                prev = None
                for t in range(T):
                    cur = front_end(t)
                    if prev is not None:
                        back_end(t - 1, *prev)
                    prev = cur
                back_end(T - 1, *prev)

    nc.compile()
    return nc


def make_in_maps(F_0, F_1, F_2, F_3, W, a, B=B_FULL, T=T_FULL, ncores=NCORES):
    BP = T * 128
    Fs = [np.asarray(x, np.float32) for x in (F_0, F_1, F_2, F_3)]
    d = Fs[0].shape[1]
    wf = np.ascontiguousarray(np.asarray(W, np.float32))
    a2 = np.ascontiguousarray(np.asarray(a, np.float32)[d:, 0]).reshape(1, d)
    eye = np.eye(128, dtype=np.float32)
    in_maps = []
    for c in range(ncores):
        m = {"w": wf, "a2row": a2, "eye": eye}
        for j in range(4):
            fj = np.zeros((BP, d), np.float32)
            fj[:B] = Fs[j][c * B:(c + 1) * B]
            m[f"f{j}"] = fj
        for r in range(3):
            q = 3 * c + r
            jj, b = q // ncores, q % ncores
            gr = np.zeros((BP, d), np.float32)
            gr[:B] = Fs[jj + 1][b * B:(b + 1) * B]
            m[f"g{r}"] = gr
        in_maps.append(m)
    return in_maps


_NC_CACHE = {}


def kernel(F_0, F_1, F_2, F_3, W, a):
    from concourse.bass_utils import run_bass_kernel_spmd
    if "nc" not in _NC_CACHE:
        _NC_CACHE["nc"] = build_kernel()
    nc = _NC_CACHE["nc"]
    in_maps = make_in_maps(F_0, F_1, F_2, F_3, W, a)
    res = run_bass_kernel_spmd(nc, in_maps, core_ids=list(range(NCORES)))
    out = np.concatenate(
        [res.results[c]["out"][:B_FULL] for c in range(NCORES)], axis=0)
    return np.ascontiguousarray(out, np.float32)


if __name__ == "__main__":
    nc = build_kernel()
    print("build ok")


# revision 11
# speedup vs baseline: 1.4825x; 1.0278x over previous
"""AdaptiveFusionLayer Trainium2 kernel — 8-core data-parallel, gather-free.

Math (derived from the reference):
  u2 = W @ a[D:]                               # [D]
  v  = concat([F_1 @ u2, F_2 @ u2, F_3 @ u2])  # [3N]  (block layout)
  e[n, :] = softmax_k(v[3n + k])               # Wh1 cancels (softmax shift-invariance)
  wh = e0*F_1 + e1*F_2 + e2*F_3                # row-wise
  out = relu(wh @ W) + F_0

Sharding: node-blocks of B = N/8 rows per core.  The score entries core c
needs, v[3Bc : 3B(c+1)), are exactly blocks q = 3c+r (r=0..2) of v, where
block q is F_{q//8 + 1}[ (q%8)·B : (q%8+1)·B ] @ u2.  The host hands core c
those three row-blocks as extra inputs g0..g2, so no collective is needed:
every core computes exactly the scores it consumes.
"""

import numpy as np

N, D, NCORES = 50000, 768, 8
B_FULL = N // NCORES           # 6250 real nodes per core
T_FULL = (B_FULL + 127) // 128  # 49 node-tiles (padded)


def build_kernel(B=B_FULL, T=T_FULL, stage=99):
    from concourse import bass, bacc, tile, mybir

    f32 = mybir.dt.float32
    bf16 = mybir.dt.bfloat16
    Alu = mybir.AluOpType
    Act = mybir.ActivationFunctionType

    BP = T * 128                # padded nodes per core
    VPAD = 3 * BP               # padded score vector length
    FULL_T = B // 128           # full 128-tiles of real nodes
    TAIL = B - FULL_T * 128     # real nodes in the last tile
    NQ = D // 128               # 6 chunks of the feature dim

    nc = bacc.Bacc(None, target_bir_lowering=False, debug=False)

    f = [nc.declare_dram_parameter(f"f{j}", [BP, D], f32, isOutput=False)
         for j in range(4)]
    g = [nc.declare_dram_parameter(f"g{r}", [BP, D], f32, isOutput=False)
         for r in range(3)]
    w = nc.declare_dram_parameter("w", [D, D], f32, isOutput=False)
    a2row = nc.declare_dram_parameter("a2row", [1, D], f32, isOutput=False)
    eye = nc.declare_dram_parameter("eye", [128, 128], f32, isOutput=False)
    out = nc.declare_dram_parameter("out", [BP, D], f32, isOutput=True)

    with tile.TileContext(nc) as tc:
        with (
            tc.tile_pool(name="const", bufs=1) as constp,
            tc.tile_pool(name="gpool", bufs=6) as gpool,
            tc.tile_pool(name="fpool", bufs=4) as fpool,
            tc.tile_pool(name="scr", bufs=2) as scrp,
            tc.tile_pool(name="whpool", bufs=3) as whpool,
            tc.tile_pool(name="psum", bufs=2, space="PSUM") as psump,
            tc.tile_pool(name="dram", bufs=1, space="DRAM") as dramp,
        ):
            def dummy_out():
                dummy = whpool.tile([128, D], f32, tag="res", name="dummy")
                nc.vector.memset(dummy[:], 0.0)
                for t in range(T):
                    nc.sync.dma_start(out[t * 128:(t + 1) * 128, :], dummy[:])

            # ---------------- setup ----------------
            w_sb = constp.tile([128, NQ * D], f32)
            for q in range(NQ):
                nc.sync.dma_start(w_sb[:, q * D:(q + 1) * D],
                                  w[q * 128:(q + 1) * 128, :])
            w_bf = constp.tile([128, NQ * D], bf16)
            for q in range(NQ):
                nc.scalar.copy(w_bf[:, q * D:(q + 1) * D],
                               w_sb[:, q * D:(q + 1) * D])
            a2_sb = constp.tile([1, D], f32)
            nc.sync.dma_start(a2_sb[:], a2row[:])
            eye_f32 = constp.tile([128, 128], f32)
            nc.sync.dma_start(eye_f32[:], eye[:])
            eye_sb = constp.tile([128, 128], bf16)
            nc.scalar.copy(eye_sb[:], eye_f32[:])
            ones_sb = constp.tile([1, 128], f32)
            nc.vector.memset(ones_sb[:], 1.0)

            # a2 broadcast across partitions via PE outer product
            bc_ps = psump.tile([128, D], f32, tag="out")
            nc.tensor.matmul(bc_ps[:, 0:512], ones_sb[:], a2_sb[:, 0:512],
                             start=True, stop=True)
            nc.tensor.matmul(bc_ps[:, 512:D], ones_sb[:], a2_sb[:, 512:D],
                             start=True, stop=True)
            a2b_sb = constp.tile([128, D], f32)
            nc.scalar.copy(a2b_sb[:], bc_ps[:, 0:D])

            # u2 = W @ a2 : chunk q on partitions -> u2col[:, q]
            u2col = constp.tile([128, NQ], f32)
            for q in range(NQ):
                ttr_scr = scrp.tile([128, D], f32, tag="ttr", name="ttr_scr")
                nc.vector.affine_mul_reduce(
                    out=ttr_scr[:], accum_out=u2col[:, q:q + 1],
                    in0=w_sb[:, q * D:(q + 1) * D], in1=a2b_sb[:],
                    scale=1.0, bias=0.0)
            # u2col [128, NQ] -> u2row [1, D]  (d = 128 q + p)
            u2row = constp.tile([1, D], f32)
            for q in range(NQ):
                nc.sync.dma_start(
                    u2row[:, q * 128:(q + 1) * 128]
                    .rearrange("o (p z) -> o p z", z=1),
                    u2col[:, q:q + 1])
            # u2 broadcast across partitions
            u2b_ps = psump.tile([128, D], f32, tag="out")
            nc.tensor.matmul(u2b_ps[:, 0:512], ones_sb[:], u2row[:, 0:512],
                             start=True, stop=True)
            nc.tensor.matmul(u2b_ps[:, 512:D], ones_sb[:], u2row[:, 512:D],
                             start=True, stop=True)
            u2b_sb = constp.tile([128, D], f32)
            nc.scalar.copy(u2b_sb[:], u2b_ps[:, 0:D])

            done = stage < 2
            if done:
                dummy_out()

            # ---------------- phase A: scores ----------------
            if not done:
                sig_sb = constp.tile([128, 3 * T], f32)   # col r*T + t
                for r in range(3):
                    for t in range(T):
                        gt = gpool.tile([128, D], f32, tag="g", name="gt")
                        nc.sync.dma_start(gt[:], g[r][t * 128:(t + 1) * 128, :])
                        ttr_scr = scrp.tile([128, D], f32, tag="ttr",
                                            name="ttr_scr")
                        nc.vector.affine_mul_reduce(
                            out=ttr_scr[:],
                            accum_out=sig_sb[:, r * T + t: r * T + t + 1],
                            in0=gt[:], in1=u2b_sb[:], scale=1.0, bias=0.0)

                # transpose scores to [t, p] and scatter contiguously:
                # vloc[r*B + 128 t + p]
                vloc = dramp.tile([VPAD], f32)
                for r in range(3):
                    sigT_ps = psump.tile([T, 128], f32, tag="out",
                                         name="sigT_ps")
                    nc.tensor.transpose(sigT_ps[:],
                                        sig_sb[:, r * T:(r + 1) * T],
                                        eye_f32[:])
                    sigT_sb = scrp.tile([T, 128], f32, tag="sigT",
                                        name="sigT_sb")
                    nc.scalar.copy(sigT_sb[:], sigT_ps[:])
                    if FULL_T:
                        nc.sync.dma_start(
                            vloc[r * B: r * B + FULL_T * 128]
                            .rearrange("(t p) -> t p", p=128),
                            sigT_sb[0:FULL_T, :])
                    if TAIL:
                        nc.sync.dma_start(
                            vloc[r * B + FULL_T * 128: (r + 1) * B]
                            .rearrange("(o p) -> o p", o=1),
                            sigT_sb[FULL_T:FULL_T + 1, 0:TAIL])
                # zero the padded tail so exp() stays finite
                if VPAD > 3 * B:
                    zpad = constp.tile([VPAD - 3 * B, 1], f32)
                    nc.vector.memset(zpad[:], 0.0)
                    nc.sync.dma_start(
                        vloc[3 * B: VPAD].rearrange("(p o) -> p o", o=1),
                        zpad[:])
                if stage < 3:
                    dummy_out()
                    done = True

            # ---------------- softmax ----------------
            if not done:
                # V2[t, c] = vloc[384 t + c]; L_k[p, t] = V2[t, 3p + k]
                V2_sb = constp.tile([T, 3 * 128], f32)
                nc.sync.dma_start(
                    V2_sb[:], vloc[:].rearrange("(t c) -> t c", c=384))
                Ps = []
                for k in range(3):
                    Lk_ps = psump.tile([128, T], f32, tag="out", name="Lk_ps")
                    nc.tensor.transpose(
                        Lk_ps[:],
                        V2_sb.rearrange("t (p k) -> t p k", k=3)[:, :, k],
                        eye_f32[0:T, 0:T])
                    Pk = constp.tile([128, T], f32, name=f"P{k}_sb",
                                     tag=f"P{k}")
                    nc.scalar.activation(Pk[:], Lk_ps[:], Act.Exp)
                    Ps.append(Pk)
                S_sb = constp.tile([128, T], f32)
                nc.vector.tensor_add(S_sb[:], Ps[0][:], Ps[1][:])
                S2_sb = constp.tile([128, T], f32)
                nc.vector.tensor_add(S2_sb[:], S_sb[:], Ps[2][:])
                R_sb = constp.tile([128, T], f32)
                nc.vector.reciprocal(R_sb[:], S2_sb[:])
                E_sb = constp.tile([128, 3 * T], f32)  # col k*T + tau
                for k in range(3):
                    nc.vector.tensor_mul(
                        E_sb[:, k * T:(k + 1) * T], Ps[k][:], R_sb[:])
                if stage < 4:
                    dummy_out()
                    done = True

            # ---------------- phase B ----------------
            if not done:
                def front_end(t):
                    f0t = fpool.tile([128, D], f32, tag="f0", name="f0t")
                    nc.scalar.dma_start(f0t[:], f[0][t * 128:(t + 1) * 128, :])
                    fb = []
                    for j in (1, 2, 3):
                        fjt = fpool.tile([128, D], f32, tag=f"f{j}",
                                         name="fjt")
                        nc.sync.dma_start(fjt[:],
                                          f[j][t * 128:(t + 1) * 128, :])
                        # e_k-weighted bf16 cast on DVE (2x mode)
                        fjb = fpool.tile([128, D], bf16, tag=f"f{j}b",
                                         name="fjb")
                        nc.vector.tensor_scalar_mul(
                            fjb[:], fjt[:],
                            E_sb[:, (j - 1) * T + t: (j - 1) * T + t + 1])
                        fb.append(fjb)
                    # wh = fb0 + fb1 + fb2 (bf16, node-major)
                    wha = whpool.tile([128, D], bf16, tag="wha")
                    nc.vector.tensor_add(wha[:], fb[0][:], fb[1][:])
                    whb = whpool.tile([128, D], bf16, tag="whb")
                    nc.vector.tensor_add(whb[:], wha[:], fb[2][:])
                    # transpose: whp[p, 128q + n] = wh[n, 128q + p]
                    whp = psump.tile([128, D], bf16, tag="wh", bufs=4)
                    for q in range(NQ):
                        nc.tensor.transpose(
                            whp[:, q * 128:(q + 1) * 128],
                            whb[:, q * 128:(q + 1) * 128], eye_sb[:])
                    whs = whpool.tile([128, D], bf16, tag="whs")
                    nc.scalar.copy(whs[:], whp[:, 0:D])
                    return f0t, whs

                def back_end(t, f0t, whs):
                    outp = psump.tile([128, D], f32, tag="out")
                    for q in range(NQ):
                        nc.tensor.matmul(
                            outp[:, 0:512], whs[:, q * 128:(q + 1) * 128],
                            w_bf[:, q * D: q * D + 512],
                            start=(q == 0), stop=(q == NQ - 1))
                    for q in range(NQ):
                        nc.tensor.matmul(
                            outp[:, 512:D], whs[:, q * 128:(q + 1) * 128],
                            w_bf[:, q * D + 512:(q + 1) * D],
                            start=(q == 0), stop=(q == NQ - 1))
                    rel = whpool.tile([128, D], f32, tag="rel")
                    nc.scalar.activation(rel[:], outp[:, 0:D], Act.Relu)
                    res = whpool.tile([128, D], f32, tag="res")
                    nc.vector.tensor_add(res[:], rel[:], f0t[:])
                    nc.scalar.dma_start(out[t * 128:(t + 1) * 128, :],
                                        res[:])

                prev = None
                for t in range(T):
                    cur = front_end(t)
                    if prev is not None:
                        back_end(t - 1, *prev)
                    prev = cur
                back_end(T - 1, *prev)

    nc.compile()
    return nc


def make_in_maps(F_0, F_1, F_2, F_3, W, a, B=B_FULL, T=T_FULL, ncores=NCORES):
    BP = T * 128
    Fs = [np.asarray(x, np.float32) for x in (F_0, F_1, F_2, F_3)]
    d = Fs[0].shape[1]
    wf = np.ascontiguousarray(np.asarray(W, np.float32))
    a2 = np.ascontiguousarray(np.asarray(a, np.float32)[d:, 0]).reshape(1, d)
    eye = np.eye(128, dtype=np.float32)
    in_maps = []
    for c in range(ncores):
        m = {"w": wf, "a2row": a2, "eye": eye}
        for j in range(4):
            fj = np.zeros((BP, d), np.float32)
            fj[:B] = Fs[j][c * B:(c + 1) * B]
            m[f"f{j}"] = fj
        for r in range(3):
            q = 3 * c + r
            jj, b = q // ncores, q % ncores
            gr = np.zeros((BP, d), np.float32)
            gr[:B] = Fs[jj + 1][b * B:(b + 1) * B]
            m[f"g{r}"] = gr
        in_maps.append(m)
    return in_maps


_NC_CACHE = {}


def kernel(F_0, F_1, F_2, F_3, W, a):
    from concourse.bass_utils import run_bass_kernel_spmd
    if "nc" not in _NC_CACHE:
        _NC_CACHE["nc"] = build_kernel()
    nc = _NC_CACHE["nc"]
    in_maps = make_in_maps(F_0, F_1, F_2, F_3, W, a)
    res = run_bass_kernel_spmd(nc, in_maps, core_ids=list(range(NCORES)))
    out = np.concatenate(
        [res.results[c]["out"][:B_FULL] for c in range(NCORES)], axis=0)
    return np.ascontiguousarray(out, np.float32)


if __name__ == "__main__":
    nc = build_kernel()
    print("build ok")


# revision 12
# speedup vs baseline: 1.5110x; 1.0192x over previous
"""AdaptiveFusionLayer Trainium2 kernel — 8-core data-parallel, gather-free.

Math (derived from the reference):
  u2 = W @ a[D:]                               # [D]
  v  = concat([F_1 @ u2, F_2 @ u2, F_3 @ u2])  # [3N]  (block layout)
  e[n, :] = softmax_k(v[3n + k])               # Wh1 cancels (softmax shift-invariance)
  wh = e0*F_1 + e1*F_2 + e2*F_3                # row-wise
  out = relu(wh @ W) + F_0

Sharding: node-blocks of B = N/8 rows per core.  The score entries core c
needs, v[3Bc : 3B(c+1)), are exactly blocks q = 3c+r (r=0..2) of v, where
block q is F_{q//8 + 1}[ (q%8)·B : (q%8+1)·B ] @ u2.  The host hands core c
those three row-blocks as extra inputs g0..g2, so no collective is needed:
every core computes exactly the scores it consumes.
"""

import numpy as np

N, D, NCORES = 50000, 768, 8
B_FULL = N // NCORES           # 6250 real nodes per core
T_FULL = (B_FULL + 127) // 128  # 49 node-tiles (padded)


def build_kernel(B=B_FULL, T=T_FULL, stage=99):
    from concourse import bass, bacc, tile, mybir

    f32 = mybir.dt.float32
    bf16 = mybir.dt.bfloat16
    Alu = mybir.AluOpType
    Act = mybir.ActivationFunctionType

    BP = T * 128                # padded nodes per core
    VPAD = 3 * BP               # padded score vector length
    FULL_T = B // 128           # full 128-tiles of real nodes
    TAIL = B - FULL_T * 128     # real nodes in the last tile
    NQ = D // 128               # 6 chunks of the feature dim

    nc = bacc.Bacc(None, target_bir_lowering=False, debug=False)

    f = [nc.declare_dram_parameter(f"f{j}", [BP, D], f32, isOutput=False)
         for j in range(4)]
    g = [nc.declare_dram_parameter(f"g{r}", [BP, D], f32, isOutput=False)
         for r in range(3)]
    w = nc.declare_dram_parameter("w", [D, D], f32, isOutput=False)
    a2row = nc.declare_dram_parameter("a2row", [1, D], f32, isOutput=False)
    eye = nc.declare_dram_parameter("eye", [128, 128], f32, isOutput=False)
    out = nc.declare_dram_parameter("out", [BP, D], f32, isOutput=True)

    with tile.TileContext(nc) as tc:
        with (
            tc.tile_pool(name="const", bufs=1) as constp,
            tc.tile_pool(name="gpool", bufs=10) as gpool,
            tc.tile_pool(name="fpool", bufs=4) as fpool,
            tc.tile_pool(name="scr", bufs=2) as scrp,
            tc.tile_pool(name="whpool", bufs=3) as whpool,
            tc.tile_pool(name="psum", bufs=2, space="PSUM") as psump,
            tc.tile_pool(name="dram", bufs=1, space="DRAM") as dramp,
        ):
            def dummy_out():
                dummy = whpool.tile([128, D], f32, tag="res", name="dummy")
                nc.vector.memset(dummy[:], 0.0)
                for t in range(T):
                    nc.sync.dma_start(out[t * 128:(t + 1) * 128, :], dummy[:])

            # ---------------- setup ----------------
            w_sb = constp.tile([128, NQ * D], f32)
            for q in range(NQ):
                nc.sync.dma_start(w_sb[:, q * D:(q + 1) * D],
                                  w[q * 128:(q + 1) * 128, :])
            w_bf = constp.tile([128, NQ * D], bf16)
            for q in range(NQ):
                nc.scalar.copy(w_bf[:, q * D:(q + 1) * D],
                               w_sb[:, q * D:(q + 1) * D])
            a2_sb = constp.tile([1, D], f32)
            nc.sync.dma_start(a2_sb[:], a2row[:])
            eye_f32 = constp.tile([128, 128], f32)
            nc.sync.dma_start(eye_f32[:], eye[:])
            eye_sb = constp.tile([128, 128], bf16)
            nc.scalar.copy(eye_sb[:], eye_f32[:])
            ones_sb = constp.tile([1, 128], f32)
            nc.vector.memset(ones_sb[:], 1.0)

            # a2 broadcast across partitions via PE outer product
            bc_ps = psump.tile([128, D], f32, tag="out")
            nc.tensor.matmul(bc_ps[:, 0:512], ones_sb[:], a2_sb[:, 0:512],
                             start=True, stop=True)
            nc.tensor.matmul(bc_ps[:, 512:D], ones_sb[:], a2_sb[:, 512:D],
                             start=True, stop=True)
            a2b_sb = constp.tile([128, D], f32)
            nc.scalar.copy(a2b_sb[:], bc_ps[:, 0:D])

            # u2 = W @ a2 : chunk q on partitions -> u2col[:, q]
            u2col = constp.tile([128, NQ], f32)
            for q in range(NQ):
                ttr_scr = scrp.tile([128, D], f32, tag="ttr", name="ttr_scr")
                nc.vector.affine_mul_reduce(
                    out=ttr_scr[:], accum_out=u2col[:, q:q + 1],
                    in0=w_sb[:, q * D:(q + 1) * D], in1=a2b_sb[:],
                    scale=1.0, bias=0.0)
            # u2col [128, NQ] -> u2row [1, D]  (d = 128 q + p)
            u2row = constp.tile([1, D], f32)
            for q in range(NQ):
                nc.sync.dma_start(
                    u2row[:, q * 128:(q + 1) * 128]
                    .rearrange("o (p z) -> o p z", z=1),
                    u2col[:, q:q + 1])
            # u2 broadcast across partitions
            u2b_ps = psump.tile([128, D], f32, tag="out")
            nc.tensor.matmul(u2b_ps[:, 0:512], ones_sb[:], u2row[:, 0:512],
                             start=True, stop=True)
            nc.tensor.matmul(u2b_ps[:, 512:D], ones_sb[:], u2row[:, 512:D],
                             start=True, stop=True)
            u2b_sb = constp.tile([128, D], f32)
            nc.scalar.copy(u2b_sb[:], u2b_ps[:, 0:D])

            done = stage < 2
            if done:
                dummy_out()

            # ---------------- phase A: scores ----------------
            if not done:
                sig_sb = constp.tile([128, 3 * T], f32)   # col r*T + t
                for r in range(3):
                    for t in range(T):
                        gt = gpool.tile([128, D], f32, tag="g", name="gt")
                        nc.sync.dma_start(gt[:], g[r][t * 128:(t + 1) * 128, :])
                        ttr_scr = scrp.tile([128, D], f32, tag="ttr",
                                            name="ttr_scr")
                        nc.vector.affine_mul_reduce(
                            out=ttr_scr[:],
                            accum_out=sig_sb[:, r * T + t: r * T + t + 1],
                            in0=gt[:], in1=u2b_sb[:], scale=1.0, bias=0.0)

                # transpose scores to [t, p] and scatter contiguously:
                # vloc[r*B + 128 t + p]
                vloc = dramp.tile([VPAD], f32)
                for r in range(3):
                    sigT_ps = psump.tile([T, 128], f32, tag="out",
                                         name="sigT_ps")
                    nc.tensor.transpose(sigT_ps[:],
                                        sig_sb[:, r * T:(r + 1) * T],
                                        eye_f32[:])
                    sigT_sb = scrp.tile([T, 128], f32, tag="sigT",
                                        name="sigT_sb")
                    nc.scalar.copy(sigT_sb[:], sigT_ps[:])
                    if FULL_T:
                        nc.sync.dma_start(
                            vloc[r * B: r * B + FULL_T * 128]
                            .rearrange("(t p) -> t p", p=128),
                            sigT_sb[0:FULL_T, :])
                    if TAIL:
                        nc.sync.dma_start(
                            vloc[r * B + FULL_T * 128: (r + 1) * B]
                            .rearrange("(o p) -> o p", o=1),
                            sigT_sb[FULL_T:FULL_T + 1, 0:TAIL])
                # zero the padded tail so exp() stays finite
                if VPAD > 3 * B:
                    zpad = constp.tile([VPAD - 3 * B, 1], f32)
                    nc.vector.memset(zpad[:], 0.0)
                    nc.sync.dma_start(
                        vloc[3 * B: VPAD].rearrange("(p o) -> p o", o=1),
                        zpad[:])
                if stage < 3:
                    dummy_out()
                    done = True

            # ---------------- softmax ----------------
            if not done:
                # V2[t, c] = vloc[384 t + c]; L_k[p, t] = V2[t, 3p + k]
                V2_sb = constp.tile([T, 3 * 128], f32)
                nc.sync.dma_start(
                    V2_sb[:], vloc[:].rearrange("(t c) -> t c", c=384))
                Ps = []
                for k in range(3):
                    Lk_ps = psump.tile([128, T], f32, tag="out", name="Lk_ps")
                    nc.tensor.transpose(
                        Lk_ps[:],
                        V2_sb.rearrange("t (p k) -> t p k", k=3)[:, :, k],
                        eye_f32[0:T, 0:T])
                    Pk = constp.tile([128, T], f32, name=f"P{k}_sb",
                                     tag=f"P{k}")
                    nc.scalar.activation(Pk[:], Lk_ps[:], Act.Exp)
                    Ps.append(Pk)
                S_sb = constp.tile([128, T], f32)
                nc.vector.tensor_add(S_sb[:], Ps[0][:], Ps[1][:])
                S2_sb = constp.tile([128, T], f32)
                nc.vector.tensor_add(S2_sb[:], S_sb[:], Ps[2][:])
                R_sb = constp.tile([128, T], f32)
                nc.vector.reciprocal(R_sb[:], S2_sb[:])
                E_sb = constp.tile([128, 3 * T], f32)  # col k*T + tau
                for k in range(3):
                    nc.vector.tensor_mul(
                        E_sb[:, k * T:(k + 1) * T], Ps[k][:], R_sb[:])
                if stage < 4:
                    dummy_out()
                    done = True

            # ---------------- phase B ----------------
            if not done:
                def front_end(t):
                    f0t = fpool.tile([128, D], f32, tag="f0", name="f0t")
                    nc.scalar.dma_start(f0t[:], f[0][t * 128:(t + 1) * 128, :])
                    fb = []
                    for j in (1, 2, 3):
                        fjt = fpool.tile([128, D], f32, tag=f"f{j}",
                                         name="fjt")
                        nc.sync.dma_start(fjt[:],
                                          f[j][t * 128:(t + 1) * 128, :])
                        # e_k-weighted bf16 cast on DVE (2x mode)
                        fjb = fpool.tile([128, D], bf16, tag=f"f{j}b",
                                         name="fjb")
                        nc.vector.tensor_scalar_mul(
                            fjb[:], fjt[:],
                            E_sb[:, (j - 1) * T + t: (j - 1) * T + t + 1])
                        fb.append(fjb)
                    # wh = fb0 + fb1 + fb2 (bf16, node-major)
                    wha = whpool.tile([128, D], bf16, tag="wha")
                    nc.vector.tensor_add(wha[:], fb[0][:], fb[1][:])
                    whb = whpool.tile([128, D], bf16, tag="whb")
                    nc.vector.tensor_add(whb[:], wha[:], fb[2][:])
                    # transpose: whp[p, 128q + n] = wh[n, 128q + p]
                    whp = psump.tile([128, D], bf16, tag="wh", bufs=4)
                    for q in range(NQ):
                        nc.tensor.transpose(
                            whp[:, q * 128:(q + 1) * 128],
                            whb[:, q * 128:(q + 1) * 128], eye_sb[:])
                    whs = whpool.tile([128, D], bf16, tag="whs")
                    nc.scalar.copy(whs[:], whp[:, 0:D])
                    return f0t, whs

                def back_end(t, f0t, whs):
                    outp = psump.tile([128, D], f32, tag="out")
                    for q in range(NQ):
                        nc.tensor.matmul(
                            outp[:, 0:512], whs[:, q * 128:(q + 1) * 128],
                            w_bf[:, q * D: q * D + 512],
                            start=(q == 0), stop=(q == NQ - 1))
                    for q in range(NQ):
                        nc.tensor.matmul(
                            outp[:, 512:D], whs[:, q * 128:(q + 1) * 128],
                            w_bf[:, q * D + 512:(q + 1) * D],
                            start=(q == 0), stop=(q == NQ - 1))
                    rel = whpool.tile([128, D], f32, tag="rel")
                    nc.scalar.activation(rel[:], outp[:, 0:D], Act.Relu)
                    res = whpool.tile([128, D], f32, tag="res")
                    nc.vector.tensor_add(res[:], rel[:], f0t[:])
                    nc.scalar.dma_start(out[t * 128:(t + 1) * 128, :],
                                        res[:])

                prev = None
                for t in range(T):
                    cur = front_end(t)
                    if prev is not None:
                        back_end(t - 1, *prev)
                    prev = cur
                back_end(T - 1, *prev)

    nc.compile()
    return nc


def make_in_maps(F_0, F_1, F_2, F_3, W, a, B=B_FULL, T=T_FULL, ncores=NCORES):
    BP = T * 128
    Fs = [np.asarray(x, np.float32) for x in (F_0, F_1, F_2, F_3)]
    d = Fs[0].shape[1]
    wf = np.ascontiguousarray(np.asarray(W, np.float32))
    a2 = np.ascontiguousarray(np.asarray(a, np.float32)[d:, 0]).reshape(1, d)
    eye = np.eye(128, dtype=np.float32)
    in_maps = []
    for c in range(ncores):
        m = {"w": wf, "a2row": a2, "eye": eye}
        for j in range(4):
            fj = np.zeros((BP, d), np.float32)
            fj[:B] = Fs[j][c * B:(c + 1) * B]
            m[f"f{j}"] = fj
        for r in range(3):
            q = 3 * c + r
            jj, b = q // ncores, q % ncores
            gr = np.zeros((BP, d), np.float32)
            gr[:B] = Fs[jj + 1][b * B:(b + 1) * B]
            m[f"g{r}"] = gr
        in_maps.append(m)
    return in_maps


_NC_CACHE = {}


def kernel(F_0, F_1, F_2, F_3, W, a):
    from concourse.bass_utils import run_bass_kernel_spmd
    if "nc" not in _NC_CACHE:
        _NC_CACHE["nc"] = build_kernel()
    nc = _NC_CACHE["nc"]
    in_maps = make_in_maps(F_0, F_1, F_2, F_3, W, a)
    res = None
    for attempt in range(3):
        try:
            res = run_bass_kernel_spmd(nc, in_maps,
                                       core_ids=list(range(NCORES)))
            break
        except Exception:
            if attempt == 2:
                raise
    out = np.concatenate(
        [res.results[c]["out"][:B_FULL] for c in range(NCORES)], axis=0)
    return np.ascontiguousarray(out, np.float32)


if __name__ == "__main__":
    nc = build_kernel()
    print("build ok")


# revision 13
# speedup vs baseline: 1.6683x; 1.1042x over previous
"""AdaptiveFusionLayer Trainium2 kernel — 8-core data-parallel, gather-free.

Math (derived from the reference):
  u2 = W @ a[D:]                               # [D]
  v  = concat([F_1 @ u2, F_2 @ u2, F_3 @ u2])  # [3N]  (block layout)
  e[n, :] = softmax_k(v[3n + k])               # Wh1 cancels (softmax shift-invariance)
  wh = e0*F_1 + e1*F_2 + e2*F_3                # row-wise
  out = relu(wh @ W) + F_0

Sharding: node-blocks of B = N/8 rows per core.  The score entries core c
needs, v[3Bc : 3B(c+1)), are exactly blocks q = 3c+r (r=0..2) of v, where
block q is F_{q//8 + 1}[ (q%8)·B : (q%8+1)·B ] @ u2.  The host hands core c
those three row-blocks as extra inputs g0..g2, so no collective is needed:
every core computes exactly the scores it consumes.
"""

import numpy as np

N, D, NCORES = 50000, 768, 8
B_FULL = N // NCORES           # 6250 real nodes per core
T_FULL = (B_FULL + 127) // 128  # 49 node-tiles (padded)


def build_kernel(B=B_FULL, T=T_FULL, stage=99):
    from concourse import bass, bacc, tile, mybir

    f32 = mybir.dt.float32
    bf16 = mybir.dt.bfloat16
    Alu = mybir.AluOpType
    Act = mybir.ActivationFunctionType

    BP = T * 128                # padded nodes per core
    VPAD = 3 * BP               # padded score vector length
    FULL_T = B // 128           # full 128-tiles of real nodes
    TAIL = B - FULL_T * 128     # real nodes in the last tile
    NQ = D // 128               # 6 chunks of the feature dim

    nc = bacc.Bacc(None, target_bir_lowering=False, debug=False)

    f = [nc.declare_dram_parameter(f"f{j}", [BP, D], f32, isOutput=False)
         for j in range(4)]
    g = [nc.declare_dram_parameter(f"g{r}", [BP, D], f32, isOutput=False)
         for r in range(3)]
    w = nc.declare_dram_parameter("w", [D, D], f32, isOutput=False)
    a2row = nc.declare_dram_parameter("a2row", [1, D], f32, isOutput=False)
    eye = nc.declare_dram_parameter("eye", [128, 128], f32, isOutput=False)
    out = nc.declare_dram_parameter("out", [BP, D], f32, isOutput=True)

    with tile.TileContext(nc) as tc:
        with (
            tc.tile_pool(name="const", bufs=1) as constp,
            tc.tile_pool(name="gpool", bufs=10) as gpool,
            tc.tile_pool(name="fpool", bufs=4) as fpool,
            tc.tile_pool(name="scr", bufs=2) as scrp,
            tc.tile_pool(name="whpool", bufs=3) as whpool,
            tc.tile_pool(name="psum", bufs=2, space="PSUM") as psump,
            tc.tile_pool(name="dram", bufs=1, space="DRAM") as dramp,
        ):
            def dummy_out():
                dummy = whpool.tile([128, D], f32, tag="res", name="dummy")
                nc.vector.memset(dummy[:], 0.0)
                for t in range(T):
                    nc.sync.dma_start(out[t * 128:(t + 1) * 128, :], dummy[:])

            # ---------------- setup ----------------
            w_sb = constp.tile([128, NQ * D], f32)
            for q in range(NQ):
                nc.sync.dma_start(w_sb[:, q * D:(q + 1) * D],
                                  w[q * 128:(q + 1) * 128, :])
            w_bf = constp.tile([128, NQ * D], bf16)
            for q in range(NQ):
                nc.scalar.copy(w_bf[:, q * D:(q + 1) * D],
                               w_sb[:, q * D:(q + 1) * D])
            a2_sb = constp.tile([1, D], f32)
            nc.sync.dma_start(a2_sb[:], a2row[:])
            eye_f32 = constp.tile([128, 128], f32)
            nc.sync.dma_start(eye_f32[:], eye[:])
            eye_sb = constp.tile([128, 128], bf16)
            nc.scalar.copy(eye_sb[:], eye_f32[:])
            ones_sb = constp.tile([1, 128], f32)
            nc.vector.memset(ones_sb[:], 1.0)

            # a2 broadcast across partitions via PE outer product
            bc_ps = psump.tile([128, D], f32, tag="out")
            nc.tensor.matmul(bc_ps[:, 0:512], ones_sb[:], a2_sb[:, 0:512],
                             start=True, stop=True)
            nc.tensor.matmul(bc_ps[:, 512:D], ones_sb[:], a2_sb[:, 512:D],
                             start=True, stop=True)
            a2b_sb = constp.tile([128, D], f32)
            nc.scalar.copy(a2b_sb[:], bc_ps[:, 0:D])

            # u2 = W @ a2 : chunk q on partitions -> u2col[:, q]
            u2col = constp.tile([128, NQ], f32)
            for q in range(NQ):
                ttr_scr = scrp.tile([128, D], f32, tag="ttr", name="ttr_scr")
                nc.vector.affine_mul_reduce(
                    out=ttr_scr[:], accum_out=u2col[:, q:q + 1],
                    in0=w_sb[:, q * D:(q + 1) * D], in1=a2b_sb[:],
                    scale=1.0, bias=0.0)
            # u2col [128, NQ] -> u2row [1, D]  (d = 128 q + p)
            u2row = constp.tile([1, D], f32)
            for q in range(NQ):
                nc.sync.dma_start(
                    u2row[:, q * 128:(q + 1) * 128]
                    .rearrange("o (p z) -> o p z", z=1),
                    u2col[:, q:q + 1])
            # u2 broadcast across partitions
            u2b_ps = psump.tile([128, D], f32, tag="out")
            nc.tensor.matmul(u2b_ps[:, 0:512], ones_sb[:], u2row[:, 0:512],
                             start=True, stop=True)
            nc.tensor.matmul(u2b_ps[:, 512:D], ones_sb[:], u2row[:, 512:D],
                             start=True, stop=True)
            u2b_sb = constp.tile([128, D], f32)
            nc.scalar.copy(u2b_sb[:], u2b_ps[:, 0:D])

            done = stage < 2
            if done:
                dummy_out()

            # ---------------- phase A: scores ----------------
            if not done:
                sig_sb = constp.tile([128, 3 * T], f32)   # col r*T + t
                for r in range(3):
                    for t in range(T):
                        gt = gpool.tile([128, D], f32, tag="g", name="gt")
                        nc.sync.dma_start(gt[:], g[r][t * 128:(t + 1) * 128, :])
                        ttr_scr = scrp.tile([128, D], f32, tag="ttr",
                                            name="ttr_scr")
                        nc.vector.affine_mul_reduce(
                            out=ttr_scr[:],
                            accum_out=sig_sb[:, r * T + t: r * T + t + 1],
                            in0=gt[:], in1=u2b_sb[:], scale=1.0, bias=0.0)

                # transpose scores to [t, p] and scatter contiguously:
                # vloc[r*B + 128 t + p]
                vloc = dramp.tile([VPAD], f32)
                for r in range(3):
                    sigT_ps = psump.tile([T, 128], f32, tag="out",
                                         name="sigT_ps")
                    nc.tensor.transpose(sigT_ps[:],
                                        sig_sb[:, r * T:(r + 1) * T],
                                        eye_f32[:])
                    sigT_sb = scrp.tile([T, 128], f32, tag="sigT",
                                        name="sigT_sb")
                    nc.scalar.copy(sigT_sb[:], sigT_ps[:])
                    if FULL_T:
                        nc.sync.dma_start(
                            vloc[r * B: r * B + FULL_T * 128]
                            .rearrange("(t p) -> t p", p=128),
                            sigT_sb[0:FULL_T, :])
                    if TAIL:
                        nc.sync.dma_start(
                            vloc[r * B + FULL_T * 128: (r + 1) * B]
                            .rearrange("(o p) -> o p", o=1),
                            sigT_sb[FULL_T:FULL_T + 1, 0:TAIL])
                # zero the padded tail so exp() stays finite
                if VPAD > 3 * B:
                    zpad = constp.tile([VPAD - 3 * B, 1], f32)
                    nc.vector.memset(zpad[:], 0.0)
                    nc.sync.dma_start(
                        vloc[3 * B: VPAD].rearrange("(p o) -> p o", o=1),
                        zpad[:])
                if stage < 3:
                    dummy_out()
                    done = True

            # ---------------- softmax ----------------
            if not done:
                # V2[t, c] = vloc[384 t + c]; L_k[p, t] = V2[t, 3p + k]
                V2_sb = constp.tile([T, 3 * 128], f32)
                nc.sync.dma_start(
                    V2_sb[:], vloc[:].rearrange("(t c) -> t c", c=384))
                Ps = []
                for k in range(3):
                    Lk_ps = psump.tile([128, T], f32, tag="out", name="Lk_ps")
                    nc.tensor.transpose(
                        Lk_ps[:],
                        V2_sb.rearrange("t (p k) -> t p k", k=3)[:, :, k],
                        eye_f32[0:T, 0:T])
                    Pk = constp.tile([128, T], f32, name=f"P{k}_sb",
                                     tag=f"P{k}")
                    nc.scalar.activation(Pk[:], Lk_ps[:], Act.Exp)
                    Ps.append(Pk)
                S_sb = constp.tile([128, T], f32)
                nc.vector.tensor_add(S_sb[:], Ps[0][:], Ps[1][:])
                S2_sb = constp.tile([128, T], f32)
                nc.vector.tensor_add(S2_sb[:], S_sb[:], Ps[2][:])
                R_sb = constp.tile([128, T], f32)
                nc.vector.reciprocal(R_sb[:], S2_sb[:])
                E_sb = constp.tile([128, 3 * T], f32)  # col k*T + tau
                for k in range(3):
                    nc.vector.tensor_mul(
                        E_sb[:, k * T:(k + 1) * T], Ps[k][:], R_sb[:])
                if stage < 4:
                    dummy_out()
                    done = True

            # ---------------- phase B ----------------
            if not done:
                def front_end(t):
                    f0t = fpool.tile([128, D], f32, tag="f0", name="f0t")
                    nc.scalar.dma_start(f0t[:], f[0][t * 128:(t + 1) * 128, :])
                    fb = []
                    for j in (1, 2, 3):
                        fjt = fpool.tile([128, D], f32, tag=f"f{j}",
                                         name="fjt")
                        nc.sync.dma_start(fjt[:],
                                          f[j][t * 128:(t + 1) * 128, :])
                        # e_k-weighted bf16 cast on DVE (2x mode)
                        fjb = fpool.tile([128, D], bf16, tag=f"f{j}b",
                                         name="fjb")
                        nc.vector.tensor_scalar_mul(
                            fjb[:], fjt[:],
                            E_sb[:, (j - 1) * T + t: (j - 1) * T + t + 1])
                        fb.append(fjb)
                    # wh = fb0 + fb1 + fb2 (bf16, node-major)
                    wha = whpool.tile([128, D], bf16, tag="wha")
                    nc.vector.tensor_add(wha[:], fb[0][:], fb[1][:])
                    whb = whpool.tile([128, D], bf16, tag="whb")
                    nc.vector.tensor_add(whb[:], wha[:], fb[2][:])
                    # transpose: whp[p, 128q + n] = wh[n, 128q + p]
                    whp = psump.tile([128, D], bf16, tag="wh", bufs=4)
                    for q in range(NQ):
                        nc.tensor.transpose(
                            whp[:, q * 128:(q + 1) * 128],
                            whb[:, q * 128:(q + 1) * 128], eye_sb[:])
                    whs = whpool.tile([128, D], bf16, tag="whs")
                    nc.scalar.copy(whs[:], whp[:, 0:D])
                    return f0t, whs

                def back_end(t, f0t, whs):
                    outp = psump.tile([128, D], f32, tag="out")
                    for q in range(NQ):
                        nc.tensor.matmul(
                            outp[:, 0:512], whs[:, q * 128:(q + 1) * 128],
                            w_bf[:, q * D: q * D + 512],
                            start=(q == 0), stop=(q == NQ - 1))
                    for q in range(NQ):
                        nc.tensor.matmul(
                            outp[:, 512:D], whs[:, q * 128:(q + 1) * 128],
                            w_bf[:, q * D + 512:(q + 1) * D],
                            start=(q == 0), stop=(q == NQ - 1))
                    rel = whpool.tile([128, D], f32, tag="rel")
                    nc.scalar.activation(rel[:], outp[:, 0:D], Act.Relu)
                    res = whpool.tile([128, D], f32, tag="res")
                    nc.vector.tensor_add(res[:], rel[:], f0t[:])
                    nc.scalar.dma_start(out[t * 128:(t + 1) * 128, :],
                                        res[:])

                prev = None
                for t in range(T):
                    cur = front_end(t)
                    if prev is not None:
                        back_end(t - 1, *prev)
                    prev = cur
                back_end(T - 1, *prev)

    nc.compile()
    return nc


def make_in_maps(F_0, F_1, F_2, F_3, W, a, B=B_FULL, T=T_FULL, ncores=NCORES):
    BP = T * 128
    Fs = [np.asarray(x, np.float32) for x in (F_0, F_1, F_2, F_3)]
    d = Fs[0].shape[1]
    wf = np.ascontiguousarray(np.asarray(W, np.float32))
    a2 = np.ascontiguousarray(np.asarray(a, np.float32)[d:, 0]).reshape(1, d)
    eye = np.eye(128, dtype=np.float32)
    in_maps = []
    for c in range(ncores):
        m = {"w": wf, "a2row": a2, "eye": eye}
        for j in range(4):
            fj = np.zeros((BP, d), np.float32)
            fj[:B] = Fs[j][c * B:(c + 1) * B]
            m[f"f{j}"] = fj
        for r in range(3):
            q = 3 * c + r
            jj, b = q // ncores, q % ncores
            gr = np.zeros((BP, d), np.float32)
            gr[:B] = Fs[jj + 1][b * B:(b + 1) * B]
            m[f"g{r}"] = gr
        in_maps.append(m)
    return in_maps


_NC_CACHE = {}


def _run_on_device(in_maps):
    from concourse.bass_utils import run_bass_kernel_spmd
    if "nc" not in _NC_CACHE:
        _NC_CACHE["nc"] = build_kernel()
    res = run_bass_kernel_spmd(_NC_CACHE["nc"], in_maps,
                               core_ids=list(range(NCORES)))
    return np.concatenate(
        [res.results[c]["out"][:B_FULL] for c in range(NCORES)], axis=0)


def _run_in_subprocess(F_0, F_1, F_2, F_3, W, a):
    """Fallback: a fresh process gets a fresh device connection, which
    recovers from a wedged (NRT_EXEC_UNIT_UNRECOVERABLE) accelerator."""
    import os
    import subprocess
    import sys
    import tempfile
    with tempfile.TemporaryDirectory() as td:
        inp = os.path.join(td, "in.npz")
        outp = os.path.join(td, "out.npy")
        np.savez(inp, F_0=F_0, F_1=F_1, F_2=F_2, F_3=F_3, W=W, a=a)
        code = (
            "import numpy as np, importlib.util, sys\n"
            f"spec = importlib.util.spec_from_file_location('k', {__file__!r})\n"
            "m = importlib.util.module_from_spec(spec)\n"
            "spec.loader.exec_module(m)\n"
            f"d = np.load({inp!r})\n"
            "o = m._run_on_device(m.make_in_maps(d['F_0'], d['F_1'], "
            "d['F_2'], d['F_3'], d['W'], d['a']))\n"
            f"np.save({outp!r}, o)\n"
        )
        subprocess.run([sys.executable, "-c", code], check=True, timeout=900)
        return np.load(outp)


def kernel(F_0, F_1, F_2, F_3, W, a):
    args = (F_0, F_1, F_2, F_3, W, a)
    try:
        return _run_on_device(make_in_maps(*args))
    except Exception:
        pass
    for attempt in range(3):
        try:
            return _run_in_subprocess(*args)
        except Exception:
            if attempt == 2:
                raise


if __name__ == "__main__":
    nc = build_kernel()
    print("build ok")
